# revision 1
# baseline (speedup 1.0000x reference)
"""Trainium2 distributed Bass kernel for AdaptiveMolecularRegressor (GNN message passing).

Strategy (8 NeuronCores):
  - Nodes partitioned into 8 graph-aligned contiguous slices (each graph fully on
    one core); edges partitioned by dst ownership, sorted by dst.
  - Layer 0: per-edge source features are pre-gathered on the host (pure input
    layout, like im2col) and streamed in with plain sequential DMA — no on-device
    indirection. Layer 1: indirect-DMA gather (one 128-row instruction per edge
    column; SWDGE fixed cost ~1.4us/instr dominates, so column count is minimized
    with per-block chunk counts shared across cores).
  - Segment-sum via one-hot matmul (128 edges on the contraction axis, 64-node
    dst-block columns as RHS); SAGE linears + BatchNorm feature-major; BN stats
    all-reduced; post-BN h0 table all-gathered (bf16) between layers.
  - Pooling: indirect gather into [graph-part x slot] layout, log-fold mean/max.
  - Head MLP replicated on all cores after all-gathering pooled features.
Self-contained: hardcoded shapes, no file reads.
"""

import math
import numpy as np
import ml_dtypes

import concourse.bass as bass
import concourse.bacc as bacc
import concourse.mybir as mybir
import concourse.tile as tile
from concourse.bass import IndirectOffsetOnAxis
from concourse.bass_utils import run_bass_kernel_spmd

F32 = mybir.dt.float32
BF16 = mybir.dt.bfloat16
I32 = mybir.dt.int32
AF = mybir.ActivationFunctionType
OP = mybir.AluOpType
AX = mybir.AxisListType
BFNP = ml_dtypes.bfloat16

# Problem constants (overridable for scaled-down sim tests)
N_NODES = 100000
N_EDGES = 3200000
N_GRAPHS = 2000
IN_DIM = 12
HID = 48
ADME = 20
EPS = 1e-5
C = 8

W = 64      # dst-block width
CMAX = 160  # max columns per batch (whole blocks)
SENT = 8_000_000  # > any row index; SENT*48 fits int32 (no offset overflow)
DEBUG = 0   # 1: add L0 debug taps

# constf (f32 [128, 256]) column map
CF_IDENT = 0      # [128, 0:128] identity f32
CF_BN = 128       # bc0,g0,be0,bc1,g1,be1,npad @ 128..134
CF_S = 135        # bn scale col
CF_T = 136        # bn shift col
CF_UC = 138       # ucorr sum/sq @138,139
CF_HDB = 144      # hb1,hg1,hbe1,hb2,hb3 @144..148
CF_SCR = 152      # scratch cols 152..175
# constb (bf16 [128, 512]) column map
CB_IOTA = 0       # [128, 0:64]
CB_IDENT = 64     # [128, 64:192]
CB_WB = 192       # wl0T,wr0T,wl1T,wr1T @192,240,288,336 (48 cols each, rows 0:F)
CB_HW1 = 384      # [116, 384:448]
CB_HW2 = 448      # [64, 448:480]
CB_HW3 = 480      # [32, 480:481]


class CFG:
    def __init__(self, nmax, gmax, nch, ofs, batches):
        self.NMAX = nmax
        self.GMAX = gmax
        self.NBLK = nmax // W
        self.nch = nch                 # [NBLK] chunks per block (common to all cores)
        self.ofs = ofs                 # [NBLK+1] column offsets
        self.NCOL = int(ofs[-1])
        self.batches = batches         # list of (b0, nblocks, c0, ncols)
        self.MAXB = max(bc[1] for bc in batches)
        self.PC = gmax // 128          # pool chunks
        self.NG = C * gmax             # head graphs (padded)
        assert nmax % 512 == 0 and gmax % 128 == 0 and self.NG % 512 == 0


def _prep(x, edge_index, batch, adme):
    gsz = np.bincount(batch, minlength=N_GRAPHS).astype(np.int64)
    goff = np.concatenate([[0], np.cumsum(gsz)])
    cuts_g = np.zeros(C + 1, np.int64)
    for c in range(1, C):
        cuts_g[c] = np.argmin(np.abs(goff - c * N_NODES / C))
    cuts_g[C] = N_GRAPHS
    assert np.all(np.diff(cuts_g) > 0)
    nstart = goff[cuts_g]
    ncnt = np.diff(nstart)
    gcnt = np.diff(cuts_g)

    NMAX = int(math.ceil(ncnt.max() / 512) * 512)
    GMAX = int(math.ceil(max(gcnt.max(), 128) / 128) * 128)
    assert gsz.max() <= 128, gsz.max()
    NBLK = NMAX // W

    src = np.asarray(edge_index[0], np.int64)
    dst = np.asarray(edge_index[1], np.int64)
    owner = np.searchsorted(nstart[1:], dst, side="right")

    arangeN = np.arange(N_NODES, dtype=np.int64)
    owner_n = np.searchsorted(nstart[1:], arangeN, side="right")
    remap = (owner_n * NMAX + arangeN - nstart[owner_n]).astype(np.int64)

    per_core = []
    cnt_all = np.zeros((C, NBLK), np.int64)
    for c in range(C):
        m = owner == c
        s = src[m]
        d = dst[m] - nstart[c]
        o = np.argsort(d, kind="stable")
        s, d = s[o], d[o]
        blk = d // W
        cnt = np.bincount(blk, minlength=NBLK)
        cnt_all[c] = cnt
        per_core.append((s, d, blk, cnt))

    # per-block chunk counts, common across cores (SPMD shares one program)
    nch = np.maximum(np.ceil(cnt_all.max(axis=0) / 128).astype(np.int64), 1)
    ofs = np.concatenate([[0], np.cumsum(nch)])
    NCOL = int(ofs[-1])

    # batches: whole blocks, at most CMAX columns each
    batches = []
    b0 = 0
    while b0 < NBLK:
        b1 = b0 + 1
        while b1 < NBLK and ofs[b1 + 1] - ofs[b0] <= CMAX:
            b1 += 1
        batches.append((b0, b1 - b0, int(ofs[b0]), int(ofs[b1] - ofs[b0])))
        b0 = b1
    assert all(bc[3] <= CMAX for bc in batches)

    cfg = CFG(NMAX, GMAX, nch, ofs, batches)

    xf = np.asarray(x, np.float32)
    x_bf = xf.astype(BFNP)
    admef = np.asarray(adme, np.float32)

    seeds = []
    for c in range(C):
        s, d, blk, cnt = per_core[c]
        off = np.concatenate([[0], np.cumsum(cnt)])
        w_in = np.arange(len(d)) - off[blk]
        k = w_in // 128
        p = w_in % 128
        col = ofs[blk] + k
        rel = np.full((128, NCOL), -1.0, np.float32)
        rel[p, col] = (d - blk * W).astype(np.float32)
        # layer-0 messages pre-gathered on host (input layout prep)
        msgs0 = np.zeros((128, NCOL, IN_DIM), BFNP)
        msgs0[p, col] = x_bf[s]
        msgs0 = np.ascontiguousarray(msgs0.reshape(128, NCOL * IN_DIM))
        # layer-1 gather rows (remapped into the all-gathered table)
        idx0 = np.zeros((128, NCOL), np.int64)
        idx0[p, col] = s
        idx1 = remap[idx0].astype(np.int32)

        indeg = np.bincount(d, minlength=NMAX).astype(np.float32)
        inv_rep = np.ascontiguousarray(
            np.tile((1.0 / np.maximum(indeg, 1.0))[None, :], (HID, 1))).astype(BFNP)

        nv = int(ncnt[c])
        xT = np.zeros((IN_DIM, NMAX), BFNP)
        xT[:, :nv] = xf[nstart[c]:nstart[c + 1]].T.astype(BFNP)

        npad = NMAX - nv
        assert npad <= 1024
        padrows = np.full((128, 8), SENT, np.int32)
        pr = np.arange(nv, NMAX, dtype=np.int32)
        padrows.reshape(-1)[: len(pr)] = pr
        npad_rep = np.full((HID, 1), float(npad), np.float32)

        PC = cfg.PC
        pidx = np.full((128, PC), NMAX, np.int32)   # graph start row (NMAX -> zero tail)
        pmask = np.zeros((128, PC * 128), BFNP)     # valid slot mask per graph
        lgsz = gsz[cuts_g[c]:cuts_g[c + 1]]
        lgst = (goff[cuts_g[c]:cuts_g[c + 1]] - nstart[c]).astype(np.int64)
        for j in range(int(gcnt[c])):
            ch, pp = divmod(j, 128)
            pidx[pp, ch] = int(lgst[j])
            pmask[pp, ch * 128: ch * 128 + int(lgsz[j])] = 1.0
        inv_n = np.zeros((PC * 128, 1), np.float32)
        inv_n[:gcnt[c], 0] = 1.0 / np.maximum(lgsz, 1)

        adme_pad = np.zeros((GMAX, ADME), np.float32)
        adme_pad[:gcnt[c]] = admef[cuts_g[c]:cuts_g[c + 1]]

        seeds.append(dict(
            msgs0=msgs0, idx1=idx1, rel=rel.astype(BFNP),
            inv_rep=inv_rep, xT=xT, padrows=padrows, npad_rep=npad_rep,
            pidx=pidx, pmask=pmask, inv_n=inv_n, adme_pad=adme_pad,
        ))

    meta = dict(cuts_g=cuts_g, gcnt=gcnt)
    return cfg, seeds, meta


def _consts():
    cf = np.zeros((128, 256), np.float32)
    cf[:, 0:128] = np.eye(128, dtype=np.float32)
    cb = np.zeros((128, 512), np.float32)
    cb[:, CB_IOTA:CB_IOTA + 64] = np.arange(W, dtype=np.float32)[None, :]
    cb[:, CB_IDENT:CB_IDENT + 128] = np.eye(128, dtype=np.float32)
    return dict(constf_init=cf, constb_init=cb.astype(BFNP))


def _weights(ws):
    out = {}
    wb = np.zeros((128, 512), np.float32)
    wb[:IN_DIM, CB_WB:CB_WB + HID] = ws["wl0"].T
    wb[:IN_DIM, CB_WB + HID:CB_WB + 2 * HID] = ws["wr0"].T
    wb[:HID, CB_WB + 2 * HID:CB_WB + 3 * HID] = ws["wl1"].T
    wb[:HID, CB_WB + 3 * HID:CB_WB + 4 * HID] = ws["wr1"].T
    wb[:2 * HID + ADME, CB_HW1:CB_HW1 + 64] = ws["hw1"].T
    wb[:64, CB_HW2:CB_HW2 + 32] = ws["hw2"].T
    wb[:32, CB_HW3:CB_HW3 + 1] = ws["hw3"].T
    out["wtail"] = wb.astype(BFNP)  # merged into constb on device

    bn = np.zeros((128, 32), np.float32)
    for i, n in enumerate(("bc0", "g0", "be0", "bc1", "g1", "be1")):
        bn[:HID, i] = np.asarray(ws[n], np.float32)
    for i, (n, dd) in enumerate((("hb1", 64), ("hg1", 64), ("hbe1", 64),
                                 ("hb2", 32), ("hb3", 1))):
        bn[:dd, 16 + i] = np.asarray(ws[n], np.float32)
    out["bntail"] = bn
    return out


def build(cfg):
    nc = bacc.Bacc("TRN2", target_bir_lowering=False, debug=False, num_devices=C)
    NMAX, GMAX, NCOL = cfg.NMAX, cfg.GMAX, cfg.NCOL
    NBLK, PC, NG = cfg.NBLK, cfg.PC, cfg.NG
    nch, ofs, batches = cfg.nch, cfg.ofs, cfg.batches
    FIN = 2 * HID + ADME
    RG = [list(range(C))]

    P = nc.declare_dram_parameter
    msgs0 = P("msgs0", [128, NCOL * IN_DIM], BF16, isOutput=False)
    idx1 = P("idx1", [128, NCOL], I32, isOutput=False)
    rel = P("rel", [128, NCOL], BF16, isOutput=False)
    inv_rep = P("inv_rep", [HID, NMAX], BF16, isOutput=False)
    xT = P("xT", [IN_DIM, NMAX], BF16, isOutput=False)
    padrows = P("padrows", [128, 8], I32, isOutput=False)
    npad_rep = P("npad_rep", [HID, 1], F32, isOutput=False)
    pidx = P("pidx", [128, PC], I32, isOutput=False)
    pmask = P("pmask", [128, PC * 128], BF16, isOutput=False)
    inv_n = P("inv_n", [PC * 128, 1], F32, isOutput=False)
    adme_pad = P("adme_pad", [GMAX, ADME], F32, isOutput=False)
    constf_init = P("constf_init", [128, 256], F32, isOutput=False)
    constb_init = P("constb_init", [128, 512], BF16, isOutput=False)
    wtail = P("wtail", [128, 512], BF16, isOutput=False)
    bntail = P("bntail", [128, 32], F32, isOutput=False)
    out_ext = P("out", [1, NG], F32, isOutput=True)
    if DEBUG:
        dbg_hpre = P("dbg_hpre", [HID, NMAX], BF16, isOutput=True)
        dbg_hpost = P("dbg_hpost", [HID, NMAX], BF16, isOutput=True)
        dbg_cf = P("dbg_cf", [128, 256], F32, isOutput=True)
        dbg_stat = P("dbg_stat", [HID, 4], F32, isOutput=True)
        dbg_acc = P("dbg_acc", [HID, 512], F32, isOutput=True)

    with tile.TileContext(nc) as tc:
        with (
            tc.tile_pool(name="dram", bufs=1, space="DRAM") as dpool,
            tc.tile_pool(name="sres", bufs=1) as sres,
        ):
            tab0_loc = dpool.tile([NMAX, HID], BF16)
            tab0_all = dpool.tile([C * NMAX, HID], BF16, addr_space="Shared")
            tab1_loc = dpool.tile([NMAX + 128, HID], BF16)
            stat_in0 = dpool.tile([HID, 2], F32)
            stat_out0 = dpool.tile([HID, 2], F32, addr_space="Shared")
            stat_in1 = dpool.tile([HID, 2], F32)
            stat_out1 = dpool.tile([HID, 2], F32, addr_space="Shared")
            pool_in = dpool.tile([GMAX, FIN], F32)
            pool_all = dpool.tile([C * GMAX, FIN], F32, addr_space="Shared")

            constf = sres.tile([128, 256], F32)
            constb = sres.tile([128, 512], BF16)
            acc = sres.tile([HID, 512], F32)  # sumacc 0:NBLK, sqacc 256:256+NBLK
            nc.sync.dma_start(out=constf[:, :], in_=constf_init[:, :])
            nc.sync.dma_start(out=constf[:, CF_BN:CF_BN + 6],
                              in_=bntail[:, 0:6])
            nc.sync.dma_start(out=constf[:, CF_HDB:CF_HDB + 5],
                              in_=bntail[:, 16:21])
            nc.sync.dma_start(out=constf[:HID, CF_BN + 6:CF_BN + 7],
                              in_=npad_rep[:, :])
            nc.sync.dma_start(out=constb[:, 0:CB_WB], in_=constb_init[:, 0:CB_WB])
            nc.sync.dma_start(out=constb[:, CB_WB:], in_=wtail[:, CB_WB:])

            def ident_f():
                return constf[:, CF_IDENT:CF_IDENT + 128]

            def bncol(i):
                return constf[:HID, CF_BN + i:CF_BN + i + 1]

            def hdbcol(i, d):
                return constf[:d, CF_HDB + i:CF_HDB + i + 1]

            def scr(i, d=HID):
                return constf[:d, CF_SCR + i:CF_SCR + i + 1]

            # ======== Layers ========
            with (
                tc.tile_pool(name="slay", bufs=1) as slay,
                tc.tile_pool(name="sio", bufs=2) as sio,
                tc.tile_pool(name="sio3", bufs=3) as sio3,
                tc.tile_pool(name="strs", bufs=1) as strs,
                tc.tile_pool(name="pmm", bufs=2, space="PSUM") as pmm,
                tc.tile_pool(name="phh", bufs=2, space="PSUM") as phh,
                tc.tile_pool(name="ptr", bufs=2, space="PSUM") as ptr,
            ):
                rel_sb = slay.tile([128, NCOL], BF16)
                hpre = slay.tile([HID, NMAX], BF16)
                hpost = slay.tile([HID, NMAX], BF16)
                stage = slay.tile([128, (NMAX // 128) * HID], BF16)
                sqtrash = strs.tile([HID, W], F32)
                nc.sync.dma_start(out=rel_sb[:], in_=rel[:])

                def sage_layer(layer):
                    Fdim = IN_DIM if layer == 0 else HID
                    wloff = CB_WB if layer == 0 else CB_WB + 2 * HID
                    for (b0, nb, c0, ncols) in batches:
                        msgs = sio3.tile([128, CMAX * Fdim], BF16, tag="msgs")
                        if layer == 0:
                            nc.sync.dma_start(
                                out=msgs[:, :ncols * Fdim],
                                in_=msgs0[:, c0 * Fdim:(c0 + ncols) * Fdim])
                            xTb = sio.tile([IN_DIM, cfg.MAXB * W], BF16, tag="xTb")
                            nc.sync.dma_start(out=xTb[:, :nb * W],
                                              in_=xT[:, b0 * W:(b0 + nb) * W])
                        else:
                            idx_sb = sio3.tile([128, CMAX], I32, tag="idx")
                            nc.sync.dma_start(out=idx_sb[:, :ncols],
                                              in_=idx1[:, c0:c0 + ncols])
                            for j in range(ncols):
                                nc.gpsimd.indirect_dma_start(
                                    out=msgs[:, j * Fdim:(j + 1) * Fdim],
                                    out_offset=None, in_=tab0_all[:],
                                    in_offset=IndirectOffsetOnAxis(
                                        ap=idx_sb[:, j:j + 1], axis=0))
                        invb = sio.tile([HID, cfg.MAXB * W], BF16, tag="invb")
                        nc.sync.dma_start(out=invb[:, :nb * W],
                                          in_=inv_rep[:, b0 * W:(b0 + nb) * W])
                        oh = sio.tile([128, CMAX * W], BF16, tag="oh")
                        nc.vector.tensor_tensor(
                            out=oh[:, :ncols * W].rearrange("p (c w) -> p c w", w=W),
                            in0=rel_sb[:, c0:c0 + ncols].unsqueeze(2)
                                .to_broadcast([128, ncols, W]),
                            in1=constb[:, CB_IOTA:CB_IOTA + W].unsqueeze(1)
                                .to_broadcast([128, ncols, W]),
                            op=OP.is_equal)
                        for bi in range(nb):
                            b = b0 + bi
                            nk = int(nch[b])
                            lc = int(ofs[b]) - c0
                            pa = pmm.tile([Fdim, W], F32, tag="agg")
                            for k in range(nk):
                                cc = lc + k
                                nc.tensor.matmul(
                                    pa[:], lhsT=msgs[:, cc * Fdim:(cc + 1) * Fdim],
                                    rhs=oh[:, cc * W:(cc + 1) * W],
                                    start=(k == 0), stop=(k == nk - 1))
                            aggT = strs.tile([Fdim, W], BF16, tag="aggT")
                            nc.vector.tensor_tensor(
                                out=aggT[:], in0=pa[:],
                                in1=invb[:Fdim, bi * W:(bi + 1) * W],
                                op=OP.mult)
                            ph2 = phh.tile([HID, W], F32, tag="hblk")
                            nc.tensor.matmul(
                                ph2[:], lhsT=constb[:Fdim, wloff:wloff + HID],
                                rhs=aggT[:], start=True, stop=False)
                            rhs2 = (xTb[:, bi * W:(bi + 1) * W]
                                    if layer == 0
                                    else hpost[:, b * W:(b + 1) * W])
                            nc.tensor.matmul(
                                ph2[:], lhsT=constb[:Fdim, wloff + HID:wloff + 2 * HID],
                                rhs=rhs2, start=False, stop=True)
                            nc.scalar.activation(
                                hpre[:, b * W:(b + 1) * W], ph2[:], AF.Identity,
                                accum_out=acc[:, b:b + 1])
                            nc.scalar.activation(
                                sqtrash[:], ph2[:], AF.Square,
                                accum_out=acc[:, 256 + b:257 + b])

                def bn_params(bce, gce, bee, corr, stat_in, stat_out):
                    ssum = scr(0)
                    nc.vector.tensor_reduce(ssum, acc[:, 0:NBLK], AX.X, OP.add)
                    ssq = scr(1)
                    nc.vector.tensor_reduce(ssq, acc[:, 256:256 + NBLK], AX.X, OP.add)
                    if corr:
                        nc.vector.tensor_tensor(ssum, ssum, constf[:HID, CF_UC:CF_UC + 1],
                                                OP.subtract)
                        nc.vector.tensor_tensor(ssq, ssq, constf[:HID, CF_UC + 1:CF_UC + 2],
                                                OP.subtract)
                    st = strs.tile([HID, 2], F32, tag="stio")
                    nc.vector.tensor_copy(st[:, 0:1], ssum)
                    nc.vector.tensor_copy(st[:, 1:2], ssq)
                    nc.sync.dma_start(out=stat_in[:], in_=st[:])
                    nc.gpsimd.collective_compute(
                        "AllReduce", OP.add, replica_groups=RG,
                        ins=[stat_in.opt()], outs=[stat_out.opt()])
                    st2 = strs.tile([HID, 2], F32, tag="stio2")
                    nc.sync.dma_start(out=st2[:], in_=stat_out[:])
                    mean = scr(2)
                    nc.vector.tensor_scalar(mean, st2[:, 0:1], 1.0 / N_NODES, None, OP.mult)
                    ex2 = scr(3)
                    nc.vector.tensor_scalar(ex2, st2[:, 1:2], 1.0 / N_NODES, None, OP.mult)
                    msq = scr(4)
                    nc.scalar.activation(msq, mean, AF.Square)
                    var = scr(5)
                    nc.vector.tensor_tensor(var, ex2, msq, OP.subtract)
                    nc.vector.tensor_scalar(var, var, EPS, None, OP.add)
                    sd = scr(6)
                    nc.scalar.activation(sd, var, AF.Sqrt)
                    rsd = scr(7)
                    nc.vector.reciprocal(rsd, sd)
                    scol = constf[:HID, CF_S:CF_S + 1]
                    tcol = constf[:HID, CF_T:CF_T + 1]
                    nc.vector.tensor_tensor(scol, gce, rsd, OP.mult)
                    tt = scr(8)
                    nc.vector.tensor_tensor(tt, mean, scol, OP.mult)
                    nc.vector.tensor_tensor(tcol, bee, tt, OP.subtract)

                def write_table(src_sb, dst_dram):
                    for i in range(NMAX // 128):
                        pt = ptr.tile([128, HID], BF16, tag="tr")
                        nc.tensor.matmul(pt[:], lhsT=src_sb[:, i * 128:(i + 1) * 128],
                                         rhs=constb[:HID, CB_IDENT:CB_IDENT + HID],
                                         is_transpose=True, start=True, stop=True)
                        nc.scalar.activation(stage[:, i * HID:(i + 1) * HID], pt[:],
                                             AF.Copy)
                    nc.sync.dma_start(
                        out=dst_dram[:].rearrange("(c p) f -> p c f", p=128)
                            [:, :NMAX // 128, :],
                        in_=stage[:].rearrange("p (c f) -> p c f", f=HID))

                # -------- Layer 0 --------
                sage_layer(0)
                bn_params(bncol(0), bncol(1), bncol(2), False, stat_in0, stat_out0)
                nc.scalar.activation(hpost[:], hpre[:], AF.Relu,
                                     bias=constf[:HID, CF_T:CF_T + 1],
                                     scale=constf[:HID, CF_S:CF_S + 1])
                if DEBUG:
                    nc.sync.dma_start(out=dbg_stat[:, 0:2], in_=stat_in0[:])
                    nc.sync.dma_start(out=dbg_stat[:, 2:4], in_=stat_out0[:])
                    nc.sync.dma_start(out=dbg_hpre[:, :], in_=hpre[:])
                    nc.sync.dma_start(out=dbg_hpost[:, :], in_=hpost[:])
                    nc.sync.dma_start(out=dbg_cf[:, :], in_=constf[:])
                    nc.sync.dma_start(out=dbg_acc[:, 0:NBLK], in_=acc[:, 0:NBLK])
                    nc.sync.dma_start(out=dbg_acc[:, 256:256 + NBLK],
                                      in_=acc[:, 256:256 + NBLK])
                write_table(hpost, tab0_loc)
                pad_sb = strs.tile([128, 8], I32, tag="padr")
                nc.sync.dma_start(out=pad_sb[:], in_=padrows[:])
                zer_bf = strs.tile([128, 8 * HID], BF16, tag="zbf")
                nc.vector.memset(zer_bf[:], 0.0)
                for j in range(8):
                    nc.gpsimd.indirect_dma_start(
                        out=tab0_loc[:],
                        out_offset=IndirectOffsetOnAxis(ap=pad_sb[:, j:j + 1], axis=0),
                        in_=zer_bf[:, j * HID:(j + 1) * HID], in_offset=None,
                        bounds_check=NMAX - 1, oob_is_err=False)
                nc.gpsimd.collective_compute(
                    "AllGather", OP.bypass, replica_groups=RG,
                    ins=[tab0_loc.opt()], outs=[tab0_all.opt()])

                # pad-column correction for L1 stats
                vpad = scr(9)
                nc.scalar.activation(vpad, constf[:HID, CF_T:CF_T + 1], AF.Relu)
                vpad_bf = strs.tile([HID, 1], BF16, tag="vpb")
                nc.vector.tensor_copy(vpad_bf[:], vpad)
                pu = ptr.tile([HID, 1], F32, tag="pu")
                nc.tensor.matmul(pu[:], lhsT=constb[:HID, CB_WB + 3 * HID:CB_WB + 4 * HID],
                                 rhs=vpad_bf[:], start=True, stop=True)
                ucol = constf[:HID, CF_UC:CF_UC + 1]
                u2col = constf[:HID, CF_UC + 1:CF_UC + 2]
                usc = scr(10)
                nc.scalar.activation(usc, pu[:], AF.Copy)
                u2t = scr(11)
                nc.scalar.activation(u2t, pu[:], AF.Square)
                nc.vector.tensor_tensor(ucol, usc, bncol(6), OP.mult)
                nc.vector.tensor_tensor(u2col, u2t, bncol(6), OP.mult)

                # -------- Layer 1 --------
                sage_layer(1)
                bn_params(bncol(3), bncol(4), bncol(5), True, stat_in1, stat_out1)
                nc.scalar.activation(hpost[:], hpre[:], AF.Relu,
                                     bias=constf[:HID, CF_T:CF_T + 1],
                                     scale=constf[:HID, CF_S:CF_S + 1])
                write_table(hpost, tab1_loc)

            # ======== Pooling ========
            with (
                tc.tile_pool(name="spool", bufs=1) as spool,
                tc.tile_pool(name="spio", bufs=2) as spio,
            ):
                pid_sb = spool.tile([128, PC], I32)
                nc.sync.dma_start(out=pid_sb[:], in_=pidx[:])
                pmask_sb = spool.tile([128, PC * 128], BF16)
                nc.sync.dma_start(out=pmask_sb[:], in_=pmask[:])
                ztail = spool.tile([128, HID], BF16)
                nc.vector.memset(ztail[:], 0.0)
                nc.sync.dma_start(
                    out=tab1_loc[:].rearrange("(c p) f -> p c f", p=128)[:, NMAX // 128:, :],
                    in_=ztail[:].rearrange("p (c f) -> p c f", f=HID))
                invn_sb = spool.tile([128, PC], F32)
                nc.sync.dma_start(out=invn_sb[:],
                                  in_=inv_n[:].rearrange("(c p) o -> p (c o)", p=128))
                adme_sb = spool.tile([128, PC * ADME], F32)
                nc.sync.dma_start(out=adme_sb[:],
                                  in_=adme_pad[:].rearrange("(c p) f -> p c f", p=128))
                pooled = spool.tile([128, PC * FIN], F32)
                for chunk in range(PC):
                    buf = spio.tile([128, 128 * HID], BF16, tag="poolbuf")
                    nc.gpsimd.indirect_dma_start(
                        out=buf[:], out_offset=None, in_=tab1_loc[:],
                        in_offset=IndirectOffsetOnAxis(
                            ap=pid_sb[:, chunk:chunk + 1], axis=0))
                    nc.vector.tensor_tensor(
                        out=buf[:].rearrange("p (s f) -> p s f", f=HID),
                        in0=buf[:].rearrange("p (s f) -> p s f", f=HID),
                        in1=pmask_sb[:, chunk * 128:(chunk + 1) * 128].unsqueeze(2)
                            .to_broadcast([128, 128, HID]),
                        op=OP.mult)
                    half = 64 * HID
                    mx = spio.tile([128, half], BF16, tag="poolmx")
                    nc.vector.tensor_tensor(mx[:], buf[:, :half], buf[:, half:], OP.max)
                    nc.vector.tensor_tensor(buf[:, :half], buf[:, :half], buf[:, half:],
                                            OP.add)
                    sz = 32 * HID
                    while sz >= HID:
                        nc.vector.tensor_tensor(mx[:, :sz], mx[:, :sz], mx[:, sz:2 * sz],
                                                OP.max)
                        nc.vector.tensor_tensor(buf[:, :sz], buf[:, :sz], buf[:, sz:2 * sz],
                                                OP.add)
                        sz //= 2
                    off = chunk * FIN
                    nc.vector.tensor_scalar(pooled[:, off:off + HID], buf[:, :HID],
                                            invn_sb[:, chunk:chunk + 1], None, OP.mult)
                    nc.vector.tensor_copy(pooled[:, off + HID:off + 2 * HID], mx[:, :HID])
                    nc.vector.tensor_copy(pooled[:, off + 2 * HID:off + FIN],
                                          adme_sb[:, chunk * ADME:(chunk + 1) * ADME])
                nc.sync.dma_start(
                    out=pool_in[:].rearrange("(c p) f -> p c f", p=128),
                    in_=pooled[:].rearrange("p (c f) -> p c f", f=FIN))
                nc.gpsimd.collective_compute(
                    "AllGather", OP.bypass, replica_groups=RG,
                    ins=[pool_in.opt()], outs=[pool_all.opt()])

            # ======== Head (replicated) ========
            with (
                tc.tile_pool(name="shd", bufs=1) as shd,
                tc.tile_pool(name="shio", bufs=2) as shio,
                tc.tile_pool(name="phd", bufs=2, space="PSUM") as phd,
            ):
                pooledT = shd.tile([FIN, NG], BF16)
                for i in range(NG // 128):
                    pch = shio.tile([128, FIN], F32, tag="pch")
                    nc.sync.dma_start(out=pch[:], in_=pool_all[i * 128:(i + 1) * 128, :])
                    pt2 = phd.tile([FIN, 128], F32, tag="trh")
                    nc.tensor.matmul(pt2[:], lhsT=pch[:], rhs=ident_f(),
                                     is_transpose=True, start=True, stop=True)
                    nc.scalar.activation(pooledT[:, i * 128:(i + 1) * 128], pt2[:],
                                         AF.Copy)
                z1 = shd.tile([64, NG], F32)
                z1acc = shd.tile([64, 16], F32)
                sqz = shd.tile([64, 512], F32)
                for i in range(NG // 512):
                    pz = phd.tile([64, 512], F32, tag="z1")
                    nc.tensor.matmul(pz[:], lhsT=constb[:FIN, CB_HW1:CB_HW1 + 64],
                                     rhs=pooledT[:, i * 512:(i + 1) * 512],
                                     start=True, stop=True)
                    nc.scalar.activation(z1[:, i * 512:(i + 1) * 512], pz[:], AF.Identity,
                                         accum_out=z1acc[:, i:i + 1])
                    nc.scalar.activation(sqz[:], pz[:], AF.Square,
                                         accum_out=z1acc[:, 8 + i:9 + i])
                zsum = scr(12, 64)
                nc.vector.tensor_reduce(zsum, z1acc[:, 0:NG // 512], AX.X, OP.add)
                zsq = scr(13, 64)
                nc.vector.tensor_reduce(zsq, z1acc[:, 8:8 + NG // 512], AX.X, OP.add)
                zmean = scr(14, 64)
                nc.vector.tensor_scalar(zmean, zsum, 1.0 / N_GRAPHS, None, OP.mult)
                zex2 = scr(15, 64)
                nc.vector.tensor_scalar(zex2, zsq, 1.0 / N_GRAPHS, None, OP.mult)
                zmsq = scr(16, 64)
                nc.scalar.activation(zmsq, zmean, AF.Square)
                zvar = scr(17, 64)
                nc.vector.tensor_tensor(zvar, zex2, zmsq, OP.subtract)
                nc.vector.tensor_scalar(zvar, zvar, EPS, None, OP.add)
                zsd = scr(18, 64)
                nc.scalar.activation(zsd, zvar, AF.Sqrt)
                zrsd = scr(19, 64)
                nc.vector.reciprocal(zrsd, zsd)
                zs = scr(20, 64)
                nc.vector.tensor_tensor(zs, hdbcol(1, 64), zrsd, OP.mult)
                zt = scr(21, 64)
                nc.vector.tensor_tensor(zt, zmean, zs, OP.mult)
                nc.vector.tensor_tensor(zt, hdbcol(2, 64), zt, OP.subtract)
                z1b = shd.tile([64, NG], BF16)
                nc.scalar.activation(z1b[:], z1[:], AF.Relu, bias=zt, scale=zs)
                z2b = shd.tile([32, NG], BF16)
                for i in range(NG // 512):
                    pz2 = phd.tile([32, 512], F32, tag="z2")
                    nc.tensor.matmul(pz2[:], lhsT=constb[:64, CB_HW2:CB_HW2 + 32],
                                     rhs=z1b[:, i * 512:(i + 1) * 512],
                                     start=True, stop=True)
                    nc.scalar.activation(z2b[:, i * 512:(i + 1) * 512], pz2[:], AF.Relu,
                                         bias=hdbcol(3, 32))
                for i in range(NG // 512):
                    pz3 = phd.tile([1, 512], F32, tag="z3")
                    nc.tensor.matmul(pz3[:], lhsT=constb[:32, CB_HW3:CB_HW3 + 1],
                                     rhs=z2b[:, i * 512:(i + 1) * 512],
                                     start=True, stop=True)
                    zch = shio.tile([1, 512], F32, tag="zch")
                    nc.vector.tensor_scalar(zch[:], pz3[:], hdbcol(4, 1), None, OP.add)
                    nc.sync.dma_start(out=out_ext[:, i * 512:(i + 1) * 512], in_=zch[:])

    nc.compile()
    return nc


def kernel(**inputs):
    x = np.asarray(inputs["x"])
    edge_index = np.asarray(inputs["edge_index"])
    batch = np.asarray(inputs["batch"])
    adme = np.asarray(inputs["adme"])
    cfg, seeds, meta = _prep(x, edge_index, batch, adme)
    consts = _consts()
    ws = _weights(inputs)
    nc = build(cfg)
    in_maps = []
    for c in range(C):
        m = dict(seeds[c])
        m.update(consts)
        m.update(ws)
        in_maps.append(m)
    res = run_bass_kernel_spmd(nc, in_maps, core_ids=list(range(C)))
    global LAST_RESULTS
    LAST_RESULTS = res
    z = res.results[0]["out"][0]
    out = np.empty(N_GRAPHS, np.float32)
    cuts_g, gcnt = meta["cuts_g"], meta["gcnt"]
    for c in range(C):
        out[cuts_g[c]:cuts_g[c + 1]] = z[c * cfg.GMAX: c * cfg.GMAX + gcnt[c]]
    return out



# revision 9
# speedup vs baseline: 1.1392x; 1.1392x over previous
"""Trainium2 distributed Bass kernel for AdaptiveMolecularRegressor (GNN message passing).

Strategy (8 NeuronCores):
  - Nodes partitioned into 8 graph-aligned contiguous slices (each graph fully on
    one core); edges partitioned by dst ownership, sorted by dst.
  - Layer 0: per-edge source features are pre-gathered on the host (pure input
    layout, like im2col) and streamed in with plain sequential DMA.
  - Layer 1: bulk SWDGE dma_gather (InstDMAGatherAnt) from an all-gathered
    256B-row table [C*NMAX, 128] bf16.  int16 gather indices reach 32768 rows,
    so each batch issues up to ceil(C*NMAX/32768) window gathers; one
    instruction moves ~10-20K rows (994ns fixed + 0.34ns/descriptor on Pool)
    instead of ~1.1us per 128 rows with per-column indirect DMA.
  - Segment-sum via one-hot matmul (128 edges on the contraction axis, 64-node
    dst-block columns as RHS); SAGE linears + BatchNorm feature-major; BN stats
    all-reduced; post-BN h0 table all-gathered between layers.
  - Pooling: indirect gather into [graph-part x slot] layout, log-fold mean/max.
  - Head MLP replicated on all cores after all-gathering pooled features.
Self-contained: hardcoded shapes, no file reads.
"""

import math
import numpy as np
import ml_dtypes

import concourse.bass as bass
import concourse.bacc as bacc
import concourse.mybir as mybir
import concourse.tile as tile
from concourse.bass import IndirectOffsetOnAxis
from concourse.bass_utils import run_bass_kernel_spmd
from concourse.library_config import mlp as LIB_MLP

F32 = mybir.dt.float32
BF16 = mybir.dt.bfloat16
I32 = mybir.dt.int32
I16 = mybir.dt.int16
AF = mybir.ActivationFunctionType
OP = mybir.AluOpType
AX = mybir.AxisListType
BFNP = ml_dtypes.bfloat16

# Problem constants
N_NODES = 100000
N_EDGES = 3200000
N_GRAPHS = 2000
IN_DIM = 12
HID = 48
ADME = 20
EPS = 1e-5
C = 8

W = 64        # dst-block width
CMAX0 = 128   # max columns per L0 batch
CMAX1 = 96    # max columns per L1 batch
WIN = 32768   # int16 gather index reach (rows)
ROWE = 128    # padded table row elems (bf16) = 256B

# constf (f32 [128, 256]) column map
CF_IDENT = 0      # [128, 0:128] identity f32
CF_BN = 128       # bc0,g0,be0,bc1,g1,be1,npad @ 128..134
CF_S = 135        # bn scale col
CF_T = 136        # bn shift col
CF_UC = 138       # ucorr sum/sq @138,139
CF_HDB = 144      # hb1,hg1,hbe1,hb2,hb3 @144..148
CF_SCR = 152      # scratch cols 152..175
# constb (bf16 [128, 512]) column map
CB_IOTA = 0       # [128, 0:64]
CB_IDENT = 64     # [128, 64:192]
CB_WB = 192       # wl0T,wr0T,wl1T,wr1T @192,240,288,336 (48 cols each, rows 0:F)
CB_HW1 = 384      # [116, 384:448]
CB_HW2 = 448      # [64, 448:480]
CB_HW3 = 480      # [32, 480:481]


class CFG:
    def __init__(self, nmax, gmax, nch0, ofs0, batches0, ncol1, batches1, nwin):
        self.NMAX = nmax
        self.GMAX = gmax
        self.NBLK = nmax // W
        self.nch0 = nch0               # [NBLK] L0 chunks per block
        self.ofs0 = ofs0               # [NBLK+1] L0 column offsets
        self.NCOL0 = int(ofs0[-1])
        self.batches0 = batches0       # list of (b0, nb, c0, ncols)
        self.NCOL1 = ncol1
        self.batches1 = batches1       # list of dicts (see _prep)
        self.NWIN = nwin
        self.MAXB = max(max(bc[1] for bc in batches0),
                        max(bt["nb"] for bt in batches1))
        self.PC = gmax // 128          # pool chunks
        self.NG = C * gmax             # head graphs (padded)
        assert nmax % 512 == 0 and gmax % 128 == 0 and self.NG % 512 == 0


def _prep(x, edge_index, batch, adme):
    gsz = np.bincount(batch, minlength=N_GRAPHS).astype(np.int64)
    goff = np.concatenate([[0], np.cumsum(gsz)])
    cuts_g = np.zeros(C + 1, np.int64)
    for c in range(1, C):
        cuts_g[c] = np.argmin(np.abs(goff - c * N_NODES / C))
    cuts_g[C] = N_GRAPHS
    assert np.all(np.diff(cuts_g) > 0)
    nstart = goff[cuts_g]
    ncnt = np.diff(nstart)
    gcnt = np.diff(cuts_g)

    NMAX = int(math.ceil(ncnt.max() / 512) * 512)
    GMAX = int(math.ceil(max(gcnt.max(), 128) / 128) * 128)
    assert gsz.max() <= 128, gsz.max()
    NBLK = NMAX // W
    NWIN = int(math.ceil(C * NMAX / WIN))

    src = np.asarray(edge_index[0], np.int64)
    dst = np.asarray(edge_index[1], np.int64)
    owner = np.searchsorted(nstart[1:], dst, side="right")

    arangeN = np.arange(N_NODES, dtype=np.int64)
    owner_n = np.searchsorted(nstart[1:], arangeN, side="right")
    remap = (owner_n * NMAX + arangeN - nstart[owner_n]).astype(np.int64)

    per_core = []
    cnt_all = np.zeros((C, NBLK), np.int64)
    cnt1_all = np.zeros((C, NBLK, NWIN), np.int64)
    for c in range(C):
        m = owner == c
        s = src[m]
        d = dst[m] - nstart[c]
        o = np.argsort(d, kind="stable")
        s, d = s[o], d[o]
        blk = d // W
        rows = remap[s]
        wv = rows // WIN
        cnt_all[c] = np.bincount(blk, minlength=NBLK)
        cnt1_all[c] = np.bincount(blk * NWIN + wv,
                                  minlength=NBLK * NWIN).reshape(NBLK, NWIN)
        per_core.append((s, d, blk, rows, wv))

    # L0: per-block chunk counts, common across cores
    nch0 = np.maximum(np.ceil(cnt_all.max(axis=0) / 128).astype(np.int64), 1)
    ofs0 = np.concatenate([[0], np.cumsum(nch0)])
    NCOL0 = int(ofs0[-1])
    batches0 = []
    b0 = 0
    while b0 < NBLK:
        b1 = b0 + 1
        while b1 < NBLK and b1 - b0 < 16 and ofs0[b1 + 1] - ofs0[b0] <= CMAX0:
            b1 += 1
        batches0.append((b0, b1 - b0, int(ofs0[b0]), int(ofs0[b1] - ofs0[b0])))
        b0 = b1
    assert all(bc[3] <= CMAX0 for bc in batches0)

    # L1: per-(block, window) chunk counts, common across cores
    nch1 = np.ceil(cnt1_all.max(axis=0) / 128).astype(np.int64)  # [NBLK, NWIN]
    for b in range(NBLK):
        if nch1[b].sum() == 0:
            nch1[b, 0] = 1
    blk_ch = nch1.sum(axis=1)
    assert blk_ch.max() <= CMAX1
    # group blocks into batches; assign columns window-major within batch
    batches1 = []
    col_bw = np.zeros((NBLK, NWIN), np.int64)
    col = 0
    b0 = 0
    while b0 < NBLK:
        b1 = b0 + 1
        tot = int(blk_ch[b0])
        while b1 < NBLK and b1 - b0 < 16 and tot + blk_ch[b1] <= CMAX1:
            tot += int(blk_ch[b1])
            b1 += 1
        c0 = col
        wruns = []
        blocks = {b: [] for b in range(b0, b1)}
        for w in range(NWIN):
            wc0 = col - c0
            wn = 0
            for b in range(b0, b1):
                nk = int(nch1[b, w])
                if nk:
                    col_bw[b, w] = col
                    blocks[b].append((col - c0, nk))
                    col += nk
                    wn += nk
            if wn:
                wruns.append((w, wc0, wn))
        batches1.append(dict(b0=b0, nb=b1 - b0, c0=c0, ncols=col - c0,
                             wruns=wruns,
                             blocks=[(b, blocks[b]) for b in range(b0, b1)]))
        b0 = b1
    NCOL1 = col
    assert all(bt["ncols"] <= CMAX1 for bt in batches1)

    cfg = CFG(NMAX, GMAX, nch0, ofs0, batches0, NCOL1, batches1, NWIN)

    xf = np.asarray(x, np.float32)
    x_bf = xf.astype(BFNP)
    admef = np.asarray(adme, np.float32)

    seeds = []
    for c in range(C):
        s, d, blk, rows, wv = per_core[c]
        # ---- L0 layout (block-major chunks) ----
        cnt = cnt_all[c]
        off = np.concatenate([[0], np.cumsum(cnt)])
        w_in = np.arange(len(d)) - off[blk]
        k = w_in // 128
        p = w_in % 128
        col0 = ofs0[blk] + k
        rel0 = np.full((128, NCOL0), -1.0, np.float32)
        rel0[p, col0] = (d - blk * W).astype(np.float32)
        msgs0 = np.zeros((128, NCOL0, IN_DIM), BFNP)
        msgs0[p, col0] = x_bf[s]
        msgs0 = np.ascontiguousarray(msgs0.reshape(128, NCOL0 * IN_DIM))

        # ---- L1 layout (window-major within batch) ----
        key = blk * NWIN + wv
        o2 = np.argsort(key, kind="stable")
        d2, blk2, rows2, wv2 = d[o2], blk[o2], rows[o2], wv[o2]
        grp = key[o2]
        gcnt1 = np.bincount(grp, minlength=NBLK * NWIN)
        goff1 = np.concatenate([[0], np.cumsum(gcnt1)])
        j1 = np.arange(len(d2)) - goff1[grp]
        k1 = j1 // 128
        p1 = j1 % 128
        col1 = col_bw[blk2, wv2] + k1
        rel1 = np.full((128, NCOL1), -1.0, np.float32)
        rel1[p1, col1] = (d2 - blk2 * W).astype(np.float32)
        idx1w = np.zeros((128, 8 * NCOL1), np.int16)
        idx1w[p1 % 16, 8 * col1 + p1 // 16] = (rows2 - wv2 * WIN).astype(np.int16)
        # replicate the 16-partition idx block across all 8 Q7-core stripes
        idx1w = np.ascontiguousarray(np.tile(idx1w[:16, :], (8, 1)))

        indeg = np.bincount(d, minlength=NMAX).astype(np.float32)
        inv_rep = np.ascontiguousarray(
            np.tile((1.0 / np.maximum(indeg, 1.0))[None, :], (HID, 1))).astype(BFNP)

        nv = int(ncnt[c])
        xT = np.zeros((IN_DIM, NMAX), BFNP)
        xT[:, :nv] = xf[nstart[c]:nstart[c + 1]].T.astype(BFNP)

        npad = NMAX - nv
        npad_rep = np.full((HID, 1), float(npad), np.float32)

        PC = cfg.PC
        pidx = np.full((128, PC), NMAX, np.int32)   # graph start row (NMAX -> zero tail)
        pmask = np.zeros((128, PC * 128), BFNP)     # valid slot mask per graph
        lgsz = gsz[cuts_g[c]:cuts_g[c + 1]]
        lgst = (goff[cuts_g[c]:cuts_g[c + 1]] - nstart[c]).astype(np.int64)
        for j in range(int(gcnt[c])):
            ch, pp = divmod(j, 128)
            pidx[pp, ch] = int(lgst[j])
            pmask[pp, ch * 128: ch * 128 + int(lgsz[j])] = 1.0
        inv_n = np.zeros((PC * 128, 1), np.float32)
        inv_n[:gcnt[c], 0] = 1.0 / np.maximum(lgsz, 1)

        adme_pad = np.zeros((GMAX, ADME), np.float32)
        adme_pad[:gcnt[c]] = admef[cuts_g[c]:cuts_g[c + 1]]

        seeds.append(dict(
            msgs0=msgs0, rel0=rel0.astype(BFNP), rel1=rel1.astype(BFNP),
            idx1w=idx1w,
            inv_rep=inv_rep, xT=xT, npad_rep=npad_rep,
            pidx=pidx, pmask=pmask, inv_n=inv_n, adme_pad=adme_pad,
        ))

    meta = dict(cuts_g=cuts_g, gcnt=gcnt)
    return cfg, seeds, meta


def _consts():
    cf = np.zeros((128, 256), np.float32)
    cf[:, 0:128] = np.eye(128, dtype=np.float32)
    cb = np.zeros((128, 512), np.float32)
    cb[:, CB_IOTA:CB_IOTA + 64] = np.arange(W, dtype=np.float32)[None, :]
    cb[:, CB_IDENT:CB_IDENT + 128] = np.eye(128, dtype=np.float32)
    return dict(constf_init=cf, constb_init=cb.astype(BFNP))


def _weights(ws):
    out = {}
    wb = np.zeros((128, 512), np.float32)
    wb[:IN_DIM, CB_WB:CB_WB + HID] = ws["wl0"].T
    wb[:IN_DIM, CB_WB + HID:CB_WB + 2 * HID] = ws["wr0"].T
    wb[:HID, CB_WB + 2 * HID:CB_WB + 3 * HID] = ws["wl1"].T
    wb[:HID, CB_WB + 3 * HID:CB_WB + 4 * HID] = ws["wr1"].T
    wb[:2 * HID + ADME, CB_HW1:CB_HW1 + 64] = ws["hw1"].T
    wb[:64, CB_HW2:CB_HW2 + 32] = ws["hw2"].T
    wb[:32, CB_HW3:CB_HW3 + 1] = ws["hw3"].T
    out["wtail"] = wb.astype(BFNP)  # merged into constb on device

    bn = np.zeros((128, 32), np.float32)
    for i, n in enumerate(("bc0", "g0", "be0", "bc1", "g1", "be1")):
        bn[:HID, i] = np.asarray(ws[n], np.float32)
    for i, (n, dd) in enumerate((("hb1", 64), ("hg1", 64), ("hbe1", 64),
                                 ("hb2", 32), ("hb3", 1))):
        bn[:dd, 16 + i] = np.asarray(ws[n], np.float32)
    out["bntail"] = bn
    return out


def build(cfg):
    nc = bacc.Bacc("TRN2", target_bir_lowering=False, debug=False, num_devices=C)
    NMAX, GMAX = cfg.NMAX, cfg.GMAX
    NCOL0, NCOL1 = cfg.NCOL0, cfg.NCOL1
    NBLK, PC, NG, NWIN = cfg.NBLK, cfg.PC, cfg.NG, cfg.NWIN
    nch0, ofs0, batches0 = cfg.nch0, cfg.ofs0, cfg.batches0
    batches1 = cfg.batches1
    FIN = 2 * HID + ADME
    RG = [list(range(C))]
    NROWS = C * NMAX

    P = nc.declare_dram_parameter
    msgs0 = P("msgs0", [128, NCOL0 * IN_DIM], BF16, isOutput=False)
    rel0 = P("rel0", [128, NCOL0], BF16, isOutput=False)
    rel1 = P("rel1", [128, NCOL1], BF16, isOutput=False)
    idx1w = P("idx1w", [128, 8 * NCOL1], I16, isOutput=False)
    inv_rep = P("inv_rep", [HID, NMAX], BF16, isOutput=False)
    xT = P("xT", [IN_DIM, NMAX], BF16, isOutput=False)
    npad_rep = P("npad_rep", [HID, 1], F32, isOutput=False)
    pidx = P("pidx", [128, PC], I32, isOutput=False)
    pmask = P("pmask", [128, PC * 128], BF16, isOutput=False)
    inv_n = P("inv_n", [PC * 128, 1], F32, isOutput=False)
    adme_pad = P("adme_pad", [GMAX, ADME], F32, isOutput=False)
    constf_init = P("constf_init", [128, 256], F32, isOutput=False)
    constb_init = P("constb_init", [128, 512], BF16, isOutput=False)
    wtail = P("wtail", [128, 512], BF16, isOutput=False)
    bntail = P("bntail", [128, 32], F32, isOutput=False)
    out_ext = P("out", [1, NG], F32, isOutput=True)

    with tile.TileContext(nc) as tc:
        with (
            tc.tile_pool(name="dram", bufs=1, space="DRAM") as dpool,
            tc.tile_pool(name="sres", bufs=1) as sres,
        ):
            tab0_loc = dpool.tile([NMAX, ROWE], BF16)
            tab0_all = dpool.tile([NROWS, ROWE], BF16, addr_space="Shared")
            tab1_loc = dpool.tile([NMAX + 128, HID], BF16)
            stat_in0 = dpool.tile([HID, 2], F32)
            stat_out0 = dpool.tile([HID, 2], F32, addr_space="Shared")
            stat_in1 = dpool.tile([HID, 2], F32)
            stat_out1 = dpool.tile([HID, 2], F32, addr_space="Shared")
            pool_in = dpool.tile([GMAX, FIN], F32)
            pool_all = dpool.tile([C * GMAX, FIN], F32, addr_space="Shared")

            nc.gpsimd.load_library(LIB_MLP)
            constf = sres.tile([128, 256], F32)
            constb = sres.tile([128, 512], BF16)
            acc = sres.tile([HID, 512], F32)  # sumacc 0:NBLK, sqacc 256:256+NBLK
            nc.sync.dma_start(out=constf[:, :], in_=constf_init[:, :])
            nc.sync.dma_start(out=constf[:, CF_BN:CF_BN + 6],
                              in_=bntail[:, 0:6])
            nc.sync.dma_start(out=constf[:, CF_HDB:CF_HDB + 5],
                              in_=bntail[:, 16:21])
            nc.sync.dma_start(out=constf[:HID, CF_BN + 6:CF_BN + 7],
                              in_=npad_rep[:, :])
            nc.sync.dma_start(out=constb[:, 0:CB_WB], in_=constb_init[:, 0:CB_WB])
            nc.sync.dma_start(out=constb[:, CB_WB:], in_=wtail[:, CB_WB:])

            def ident_f():
                return constf[:, CF_IDENT:CF_IDENT + 128]

            def bncol(i):
                return constf[:HID, CF_BN + i:CF_BN + i + 1]

            def hdbcol(i, d):
                return constf[:d, CF_HDB + i:CF_HDB + i + 1]

            def scr(i, d=HID):
                return constf[:d, CF_SCR + i:CF_SCR + i + 1]

            # ======== Layers ========
            with (
                tc.tile_pool(name="slay", bufs=1) as slay,
                tc.tile_pool(name="sio", bufs=2) as sio,
                tc.tile_pool(name="sio3", bufs=3) as sio3,
                tc.tile_pool(name="siom", bufs=2) as siom,
                tc.tile_pool(name="strs", bufs=1) as strs,
                tc.tile_pool(name="pmm", bufs=2, space="PSUM") as pmm,
                tc.tile_pool(name="phh", bufs=2, space="PSUM") as phh,
                tc.tile_pool(name="ptr", bufs=2, space="PSUM") as ptr,
            ):
                rel0_sb = slay.tile([128, NCOL0], BF16)
                rel1_sb = slay.tile([128, NCOL1], BF16)
                hpre = slay.tile([HID, NMAX], BF16)
                hpost = slay.tile([HID, NMAX], BF16)
                stage = slay.tile([128, (NMAX // 128) * ROWE], BF16)
                sqtrash = strs.tile([HID, W], F32)
                nc.sync.dma_start(out=rel0_sb[:], in_=rel0[:])
                nc.sync.dma_start(out=rel1_sb[:], in_=rel1[:])
                nc.vector.memset(stage[:], 0.0)

                def block_tail(layer, b, chain, msgs_sb, oh, invb, xTb, b_base):
                    """Aggregate chunks of block b and run the SAGE linears."""
                    Fdim = IN_DIM if layer == 0 else HID
                    estep = IN_DIM if layer == 0 else ROWE
                    wloff = CB_WB if layer == 0 else CB_WB + 2 * HID
                    bi = b - b_base
                    pa = pmm.tile([Fdim, W], F32, tag="agg")
                    nk = len(chain)
                    for i, cc in enumerate(chain):
                        nc.tensor.matmul(
                            pa[:], lhsT=msgs_sb[:, cc * estep:cc * estep + Fdim],
                            rhs=oh[:, cc * W:(cc + 1) * W],
                            start=(i == 0), stop=(i == nk - 1))
                    aggT = strs.tile([Fdim, W], BF16, tag="aggT")
                    nc.vector.tensor_tensor(
                        out=aggT[:], in0=pa[:],
                        in1=invb[:Fdim, bi * W:(bi + 1) * W],
                        op=OP.mult)
                    ph2 = phh.tile([HID, W], F32, tag="hblk")
                    nc.tensor.matmul(
                        ph2[:], lhsT=constb[:Fdim, wloff:wloff + HID],
                        rhs=aggT[:], start=True, stop=False)
                    rhs2 = (xTb[:, bi * W:(bi + 1) * W]
                            if layer == 0
                            else hpost[:, b * W:(b + 1) * W])
                    nc.tensor.matmul(
                        ph2[:], lhsT=constb[:Fdim, wloff + HID:wloff + 2 * HID],
                        rhs=rhs2, start=False, stop=True)
                    nc.scalar.activation(
                        hpre[:, b * W:(b + 1) * W], ph2[:], AF.Identity,
                        accum_out=acc[:, b:b + 1])
                    nc.scalar.activation(
                        sqtrash[:], ph2[:], AF.Square,
                        accum_out=acc[:, 256 + b:257 + b])

                def build_oh(rel_sb, c0, ncols, cmax):
                    oh = sio.tile([128, max(CMAX0, CMAX1) * W], BF16, tag="oh")
                    nc.vector.tensor_tensor(
                        out=oh[:, :ncols * W].rearrange("p (c w) -> p c w", w=W),
                        in0=rel_sb[:, c0:c0 + ncols].unsqueeze(2)
                            .to_broadcast([128, ncols, W]),
                        in1=constb[:, CB_IOTA:CB_IOTA + W].unsqueeze(1)
                            .to_broadcast([128, ncols, W]),
                        op=OP.is_equal)
                    return oh

                def sage_layer0():
                    for (b0, nb, c0, ncols) in batches0:
                        msgs = sio3.tile([128, CMAX0 * IN_DIM], BF16, tag="msgs")
                        nc.sync.dma_start(
                            out=msgs[:, :ncols * IN_DIM],
                            in_=msgs0[:, c0 * IN_DIM:(c0 + ncols) * IN_DIM])
                        xTb = sio.tile([IN_DIM, cfg.MAXB * W], BF16, tag="xTb")
                        nc.sync.dma_start(out=xTb[:, :nb * W],
                                          in_=xT[:, b0 * W:(b0 + nb) * W])
                        invb = sio.tile([HID, cfg.MAXB * W], BF16, tag="invb")
                        nc.sync.dma_start(out=invb[:, :nb * W],
                                          in_=inv_rep[:, b0 * W:(b0 + nb) * W])
                        oh = build_oh(rel0_sb, c0, ncols, CMAX0)
                        for bi in range(nb):
                            b = b0 + bi
                            lc = int(ofs0[b]) - c0
                            chain = list(range(lc, lc + int(nch0[b])))
                            block_tail(0, b, chain, msgs, oh, invb, xTb, b0)

                def sage_layer1():
                    for bt in batches1:
                        b0, nb, c0, ncols = bt["b0"], bt["nb"], bt["c0"], bt["ncols"]
                        idx_sb = sio.tile([128, 8 * CMAX1], I16, tag="idx")
                        nc.sync.dma_start(out=idx_sb[:, :8 * ncols],
                                          in_=idx1w[:, 8 * c0:8 * (c0 + ncols)])
                        msgs = siom.tile([128, CMAX1 * ROWE], BF16, tag="msgs1")
                        for (w, wc0, wn) in bt["wruns"]:
                            r0 = w * WIN
                            r1 = min(r0 + WIN, NROWS)
                            # HW caps one gather at 1024 idxs (8 columns)
                            for g0 in range(0, wn, 8):
                                gn = min(8, wn - g0)
                                ca = wc0 + g0
                                nc.gpsimd.dma_gather(
                                    out_ap=msgs[:, ca * ROWE:(ca + gn) * ROWE]
                                        .rearrange("p (c e) -> p c e", e=ROWE),
                                    in_ap=tab0_all[r0:r1, :],
                                    idxs_ap=idx_sb[:, 8 * ca:8 * (ca + gn)],
                                    num_idxs=128 * gn,
                                    num_idxs_reg=128 * gn,
                                    elem_size=ROWE)
                        invb = sio.tile([HID, cfg.MAXB * W], BF16, tag="invb")
                        nc.sync.dma_start(out=invb[:, :nb * W],
                                          in_=inv_rep[:, b0 * W:(b0 + nb) * W])
                        oh = build_oh(rel1_sb, c0, ncols, CMAX1)
                        for (b, runs) in bt["blocks"]:
                            chain = [st + k for (st, nk) in runs for k in range(nk)]
                            block_tail(1, b, chain, msgs, oh, invb, None, b0)

                def bn_params(gce, bee, corr, stat_in, stat_out):
                    ssum = scr(0)
                    nc.vector.tensor_reduce(ssum, acc[:, 0:NBLK], AX.X, OP.add)
                    ssq = scr(1)
                    nc.vector.tensor_reduce(ssq, acc[:, 256:256 + NBLK], AX.X, OP.add)
                    if corr:
                        nc.vector.tensor_tensor(ssum, ssum, constf[:HID, CF_UC:CF_UC + 1],
                                                OP.subtract)
                        nc.vector.tensor_tensor(ssq, ssq, constf[:HID, CF_UC + 1:CF_UC + 2],
                                                OP.subtract)
                    st = strs.tile([HID, 2], F32, tag="stio")
                    nc.vector.tensor_copy(st[:, 0:1], ssum)
                    nc.vector.tensor_copy(st[:, 1:2], ssq)
                    nc.sync.dma_start(out=stat_in[:], in_=st[:])
                    nc.gpsimd.collective_compute(
                        "AllReduce", OP.add, replica_groups=RG,
                        ins=[stat_in.opt()], outs=[stat_out.opt()])
                    st2 = strs.tile([HID, 2], F32, tag="stio2")
                    nc.sync.dma_start(out=st2[:], in_=stat_out[:])
                    mean = scr(2)
                    nc.vector.tensor_scalar(mean, st2[:, 0:1], 1.0 / N_NODES, None, OP.mult)
                    ex2 = scr(3)
                    nc.vector.tensor_scalar(ex2, st2[:, 1:2], 1.0 / N_NODES, None, OP.mult)
                    msq = scr(4)
                    nc.scalar.activation(msq, mean, AF.Square)
                    var = scr(5)
                    nc.vector.tensor_tensor(var, ex2, msq, OP.subtract)
                    nc.vector.tensor_scalar(var, var, EPS, None, OP.add)
                    sd = scr(6)
                    nc.scalar.activation(sd, var, AF.Sqrt)
                    rsd = scr(7)
                    nc.vector.reciprocal(rsd, sd)
                    scol = constf[:HID, CF_S:CF_S + 1]
                    tcol = constf[:HID, CF_T:CF_T + 1]
                    nc.vector.tensor_tensor(scol, gce, rsd, OP.mult)
                    tt = scr(8)
                    nc.vector.tensor_tensor(tt, mean, scol, OP.mult)
                    nc.vector.tensor_tensor(tcol, bee, tt, OP.subtract)

                def write_table(src_sb, dst_dram, rowe):
                    for i in range(NMAX // 128):
                        pt = ptr.tile([128, HID], BF16, tag="tr")
                        nc.tensor.matmul(pt[:], lhsT=src_sb[:, i * 128:(i + 1) * 128],
                                         rhs=constb[:HID, CB_IDENT:CB_IDENT + HID],
                                         is_transpose=True, start=True, stop=True)
                        nc.scalar.activation(stage[:, i * rowe:i * rowe + HID], pt[:],
                                             AF.Copy)
                    nc.sync.dma_start(
                        out=dst_dram[:].rearrange("(c p) f -> p c f", p=128)
                            [:, :NMAX // 128, :],
                        in_=stage[:, :(NMAX // 128) * rowe]
                            .rearrange("p (c f) -> p c f", f=rowe))

                # -------- Layer 0 --------
                sage_layer0()
                bn_params(bncol(1), bncol(2), False, stat_in0, stat_out0)
                nc.scalar.activation(hpost[:], hpre[:], AF.Relu,
                                     bias=constf[:HID, CF_T:CF_T + 1],
                                     scale=constf[:HID, CF_S:CF_S + 1])
                write_table(hpost, tab0_loc, ROWE)
                nc.gpsimd.collective_compute(
                    "AllGather", OP.bypass, replica_groups=RG,
                    ins=[tab0_loc.opt()], outs=[tab0_all.opt()])

                # pad-column correction for L1 stats
                vpad = scr(9)
                nc.scalar.activation(vpad, constf[:HID, CF_T:CF_T + 1], AF.Relu)
                vpad_bf = strs.tile([HID, 1], BF16, tag="vpb")
                nc.vector.tensor_copy(vpad_bf[:], vpad)
                pu = ptr.tile([HID, 1], F32, tag="pu")
                nc.tensor.matmul(pu[:], lhsT=constb[:HID, CB_WB + 3 * HID:CB_WB + 4 * HID],
                                 rhs=vpad_bf[:], start=True, stop=True)
                ucol = constf[:HID, CF_UC:CF_UC + 1]
                u2col = constf[:HID, CF_UC + 1:CF_UC + 2]
                usc = scr(10)
                nc.scalar.activation(usc, pu[:], AF.Copy)
                u2t = scr(11)
                nc.scalar.activation(u2t, pu[:], AF.Square)
                nc.vector.tensor_tensor(ucol, usc, bncol(6), OP.mult)
                nc.vector.tensor_tensor(u2col, u2t, bncol(6), OP.mult)

                # -------- Layer 1 --------
                sage_layer1()
                bn_params(bncol(4), bncol(5), True, stat_in1, stat_out1)
                nc.scalar.activation(hpost[:], hpre[:], AF.Relu,
                                     bias=constf[:HID, CF_T:CF_T + 1],
                                     scale=constf[:HID, CF_S:CF_S + 1])
                write_table(hpost, tab1_loc, HID)

            # ======== Pooling ========
            with (
                tc.tile_pool(name="spool", bufs=1) as spool,
                tc.tile_pool(name="spio", bufs=2) as spio,
            ):
                pid_sb = spool.tile([128, PC], I32)
                nc.sync.dma_start(out=pid_sb[:], in_=pidx[:])
                pmask_sb = spool.tile([128, PC * 128], BF16)
                nc.sync.dma_start(out=pmask_sb[:], in_=pmask[:])
                ztail = spool.tile([128, HID], BF16)
                nc.vector.memset(ztail[:], 0.0)
                nc.sync.dma_start(
                    out=tab1_loc[:].rearrange("(c p) f -> p c f", p=128)[:, NMAX // 128:, :],
                    in_=ztail[:].rearrange("p (c f) -> p c f", f=HID))
                invn_sb = spool.tile([128, PC], F32)
                nc.sync.dma_start(out=invn_sb[:],
                                  in_=inv_n[:].rearrange("(c p) o -> p (c o)", p=128))
                adme_sb = spool.tile([128, PC * ADME], F32)
                nc.sync.dma_start(out=adme_sb[:],
                                  in_=adme_pad[:].rearrange("(c p) f -> p c f", p=128))
                pooled = spool.tile([128, PC * FIN], F32)
                for chunk in range(PC):
                    buf = spio.tile([128, 128 * HID], BF16, tag="poolbuf")
                    nc.gpsimd.indirect_dma_start(
                        out=buf[:], out_offset=None, in_=tab1_loc[:],
                        in_offset=IndirectOffsetOnAxis(
                            ap=pid_sb[:, chunk:chunk + 1], axis=0))
                    nc.vector.tensor_tensor(
                        out=buf[:].rearrange("p (s f) -> p s f", f=HID),
                        in0=buf[:].rearrange("p (s f) -> p s f", f=HID),
                        in1=pmask_sb[:, chunk * 128:(chunk + 1) * 128].unsqueeze(2)
                            .to_broadcast([128, 128, HID]),
                        op=OP.mult)
                    half = 64 * HID
                    mx = spio.tile([128, half], BF16, tag="poolmx")
                    nc.vector.tensor_tensor(mx[:], buf[:, :half], buf[:, half:], OP.max)
                    nc.vector.tensor_tensor(buf[:, :half], buf[:, :half], buf[:, half:],
                                            OP.add)
                    sz = 32 * HID
                    while sz >= HID:
                        nc.vector.tensor_tensor(mx[:, :sz], mx[:, :sz], mx[:, sz:2 * sz],
                                                OP.max)
                        nc.vector.tensor_tensor(buf[:, :sz], buf[:, :sz], buf[:, sz:2 * sz],
                                                OP.add)
                        sz //= 2
                    off = chunk * FIN
                    nc.vector.tensor_scalar(pooled[:, off:off + HID], buf[:, :HID],
                                            invn_sb[:, chunk:chunk + 1], None, OP.mult)
                    nc.vector.tensor_copy(pooled[:, off + HID:off + 2 * HID], mx[:, :HID])
                    nc.vector.tensor_copy(pooled[:, off + 2 * HID:off + FIN],
                                          adme_sb[:, chunk * ADME:(chunk + 1) * ADME])
                nc.sync.dma_start(
                    out=pool_in[:].rearrange("(c p) f -> p c f", p=128),
                    in_=pooled[:].rearrange("p (c f) -> p c f", f=FIN))
                nc.gpsimd.collective_compute(
                    "AllGather", OP.bypass, replica_groups=RG,
                    ins=[pool_in.opt()], outs=[pool_all.opt()])

            # ======== Head (replicated) ========
            with (
                tc.tile_pool(name="shd", bufs=1) as shd,
                tc.tile_pool(name="shio", bufs=2) as shio,
                tc.tile_pool(name="phd", bufs=2, space="PSUM") as phd,
            ):
                pooledT = shd.tile([FIN, NG], BF16)
                for i in range(NG // 128):
                    pch = shio.tile([128, FIN], F32, tag="pch")
                    nc.sync.dma_start(out=pch[:], in_=pool_all[i * 128:(i + 1) * 128, :])
                    pt2 = phd.tile([FIN, 128], F32, tag="trh")
                    nc.tensor.matmul(pt2[:], lhsT=pch[:], rhs=ident_f(),
                                     is_transpose=True, start=True, stop=True)
                    nc.scalar.activation(pooledT[:, i * 128:(i + 1) * 128], pt2[:],
                                         AF.Copy)
                z1 = shd.tile([64, NG], F32)
                z1acc = shd.tile([64, 16], F32)
                sqz = shd.tile([64, 512], F32)
                for i in range(NG // 512):
                    pz = phd.tile([64, 512], F32, tag="z1")
                    nc.tensor.matmul(pz[:], lhsT=constb[:FIN, CB_HW1:CB_HW1 + 64],
                                     rhs=pooledT[:, i * 512:(i + 1) * 512],
                                     start=True, stop=True)
                    nc.scalar.activation(z1[:, i * 512:(i + 1) * 512], pz[:], AF.Identity,
                                         accum_out=z1acc[:, i:i + 1])
                    nc.scalar.activation(sqz[:], pz[:], AF.Square,
                                         accum_out=z1acc[:, 8 + i:9 + i])
                zsum = scr(12, 64)
                nc.vector.tensor_reduce(zsum, z1acc[:, 0:NG // 512], AX.X, OP.add)
                zsq = scr(13, 64)
                nc.vector.tensor_reduce(zsq, z1acc[:, 8:8 + NG // 512], AX.X, OP.add)
                zmean = scr(14, 64)
                nc.vector.tensor_scalar(zmean, zsum, 1.0 / N_GRAPHS, None, OP.mult)
                zex2 = scr(15, 64)
                nc.vector.tensor_scalar(zex2, zsq, 1.0 / N_GRAPHS, None, OP.mult)
                zmsq = scr(16, 64)
                nc.scalar.activation(zmsq, zmean, AF.Square)
                zvar = scr(17, 64)
                nc.vector.tensor_tensor(zvar, zex2, zmsq, OP.subtract)
                nc.vector.tensor_scalar(zvar, zvar, EPS, None, OP.add)
                zsd = scr(18, 64)
                nc.scalar.activation(zsd, zvar, AF.Sqrt)
                zrsd = scr(19, 64)
                nc.vector.reciprocal(zrsd, zsd)
                zs = scr(20, 64)
                nc.vector.tensor_tensor(zs, hdbcol(1, 64), zrsd, OP.mult)
                zt = scr(21, 64)
                nc.vector.tensor_tensor(zt, zmean, zs, OP.mult)
                nc.vector.tensor_tensor(zt, hdbcol(2, 64), zt, OP.subtract)
                z1b = shd.tile([64, NG], BF16)
                nc.scalar.activation(z1b[:], z1[:], AF.Relu, bias=zt, scale=zs)
                z2b = shd.tile([32, NG], BF16)
                for i in range(NG // 512):
                    pz2 = phd.tile([32, 512], F32, tag="z2")
                    nc.tensor.matmul(pz2[:], lhsT=constb[:64, CB_HW2:CB_HW2 + 32],
                                     rhs=z1b[:, i * 512:(i + 1) * 512],
                                     start=True, stop=True)
                    nc.scalar.activation(z2b[:, i * 512:(i + 1) * 512], pz2[:], AF.Relu,
                                         bias=hdbcol(3, 32))
                for i in range(NG // 512):
                    pz3 = phd.tile([1, 512], F32, tag="z3")
                    nc.tensor.matmul(pz3[:], lhsT=constb[:32, CB_HW3:CB_HW3 + 1],
                                     rhs=z2b[:, i * 512:(i + 1) * 512],
                                     start=True, stop=True)
                    zch = shio.tile([1, 512], F32, tag="zch")
                    nc.vector.tensor_scalar(zch[:], pz3[:], hdbcol(4, 1), None, OP.add)
                    nc.sync.dma_start(out=out_ext[:, i * 512:(i + 1) * 512], in_=zch[:])

    nc.compile()
    return nc


def kernel(**inputs):
    x = np.asarray(inputs["x"])
    edge_index = np.asarray(inputs["edge_index"])
    batch = np.asarray(inputs["batch"])
    adme = np.asarray(inputs["adme"])
    cfg, seeds, meta = _prep(x, edge_index, batch, adme)
    consts = _consts()
    ws = _weights(inputs)
    nc = build(cfg)
    in_maps = []
    for c in range(C):
        m = dict(seeds[c])
        m.update(consts)
        m.update(ws)
        in_maps.append(m)
    res = run_bass_kernel_spmd(nc, in_maps, core_ids=list(range(C)))
    global LAST_RESULTS
    LAST_RESULTS = res
    z = res.results[0]["out"][0]
    out = np.empty(N_GRAPHS, np.float32)
    cuts_g, gcnt = meta["cuts_g"], meta["gcnt"]
    for c in range(C):
        out[cuts_g[c]:cuts_g[c + 1]] = z[c * cfg.GMAX: c * cfg.GMAX + gcnt[c]]
    return out


# revision 12
# speedup vs baseline: 1.7028x; 1.4947x over previous
"""Trainium2 distributed Bass kernel for AdaptiveMolecularRegressor (GNN message passing).

Strategy (8 NeuronCores):
  - Nodes partitioned into 8 graph-aligned contiguous slices (each graph fully on
    one core); edges partitioned by dst ownership, sorted by dst.
  - Layer 0: per-edge source features are pre-gathered on the host (pure input
    layout, like im2col) and streamed in with plain sequential DMA.
  - Layer 1: bulk SWDGE dma_gather (InstDMAGatherAnt) from an all-gathered
    256B-row table [C*NMAX, 128] bf16.  int16 gather indices reach 32768 rows,
    so each batch issues up to ceil(C*NMAX/32768) window gathers; one
    instruction moves ~10-20K rows (994ns fixed + 0.34ns/descriptor on Pool)
    instead of ~1.1us per 128 rows with per-column indirect DMA.
  - Segment-sum via one-hot matmul (128 edges on the contraction axis, 64-node
    dst-block columns as RHS); SAGE linears + BatchNorm feature-major; BN stats
    all-reduced; post-BN h0 table all-gathered between layers.
  - Pooling: indirect gather into [graph-part x slot] layout, log-fold mean/max.
  - Head MLP replicated on all cores after all-gathering pooled features.
Self-contained: hardcoded shapes, no file reads.
"""

import math
import numpy as np
import ml_dtypes

import concourse.bass as bass
import concourse.bacc as bacc
import concourse.mybir as mybir
import concourse.tile as tile
from concourse.bass import IndirectOffsetOnAxis
from concourse.bass_utils import run_bass_kernel_spmd
from concourse.library_config import mlp as LIB_MLP

F32 = mybir.dt.float32
BF16 = mybir.dt.bfloat16
I32 = mybir.dt.int32
I16 = mybir.dt.int16
AF = mybir.ActivationFunctionType
OP = mybir.AluOpType
AX = mybir.AxisListType
BFNP = ml_dtypes.bfloat16

# Problem constants
N_NODES = 100000
N_EDGES = 3200000
N_GRAPHS = 2000
IN_DIM = 12
HID = 48
ADME = 20
EPS = 1e-5
C = 8

W = 64        # dst-block width
CMAX0 = 128   # max columns per L0 batch
CMAX1 = 96    # max columns per L1 batch
WIN = 32768   # int16 gather index reach (rows)
ROWE = 128    # padded table row elems (bf16) = 256B

# constf (f32 [128, 256]) column map
CF_IDENT = 0      # [128, 0:128] identity f32
CF_BN = 128       # bc0,g0,be0,bc1,g1,be1,npad @ 128..134
CF_S = 135        # bn scale col
CF_T = 136        # bn shift col
CF_UC = 138       # ucorr sum/sq @138,139
CF_HDB = 144      # hb1,hg1,hbe1,hb2,hb3 @144..148
CF_SCR = 152      # scratch cols 152..175
# constb (bf16 [128, 512]) column map
CB_IOTA = 0       # [128, 0:64]
CB_IDENT = 64     # [128, 64:192]
CB_WB = 192       # wl0T,wr0T,wl1T,wr1T @192,240,288,336 (48 cols each, rows 0:F)
CB_HW1 = 384      # [116, 384:448]
CB_HW2 = 448      # [64, 448:480]
CB_HW3 = 480      # [32, 480:481]


class CFG:
    def __init__(self, nmax, gmax, nch0, ofs0, batches0, ncol1, batches1, nwin):
        self.NMAX = nmax
        self.GMAX = gmax
        self.NBLK = nmax // W
        self.nch0 = nch0               # [NBLK] L0 chunks per block
        self.ofs0 = ofs0               # [NBLK+1] L0 column offsets
        self.NCOL0 = int(ofs0[-1])
        self.batches0 = batches0       # list of (b0, nb, c0, ncols)
        self.NCOL1 = ncol1
        self.batches1 = batches1       # list of dicts (see _prep)
        self.NWIN = nwin
        self.MAXB = max(max(bc[1] for bc in batches0),
                        max(bt["nb"] for bt in batches1))
        self.PC = gmax // 128          # pool chunks
        self.NG = C * gmax             # head graphs (padded)
        assert nmax % 512 == 0 and gmax % 128 == 0 and self.NG % 512 == 0


def _prep(x, edge_index, batch, adme):
    gsz = np.bincount(batch, minlength=N_GRAPHS).astype(np.int64)
    goff = np.concatenate([[0], np.cumsum(gsz)])
    cuts_g = np.zeros(C + 1, np.int64)
    for c in range(1, C):
        cuts_g[c] = np.argmin(np.abs(goff - c * N_NODES / C))
    cuts_g[C] = N_GRAPHS
    assert np.all(np.diff(cuts_g) > 0)
    nstart = goff[cuts_g]
    ncnt = np.diff(nstart)
    gcnt = np.diff(cuts_g)

    NMAX = int(math.ceil(ncnt.max() / 512) * 512)
    GMAX = int(math.ceil(max(gcnt.max(), 128) / 128) * 128)
    assert gsz.max() <= 128, gsz.max()
    NBLK = NMAX // W
    NWIN = int(math.ceil(C * NMAX / WIN))

    src = np.asarray(edge_index[0], np.int64)
    dst = np.asarray(edge_index[1], np.int64)
    owner = np.searchsorted(nstart[1:], dst, side="right")

    arangeN = np.arange(N_NODES, dtype=np.int64)
    owner_n = np.searchsorted(nstart[1:], arangeN, side="right")
    remap = (owner_n * NMAX + arangeN - nstart[owner_n]).astype(np.int64)

    per_core = []
    cnt_all = np.zeros((C, NBLK), np.int64)
    cnt1_all = np.zeros((C, NBLK, NWIN), np.int64)
    for c in range(C):
        m = owner == c
        s = src[m]
        d = dst[m] - nstart[c]
        o = np.argsort(d, kind="stable")
        s, d = s[o], d[o]
        blk = d // W
        rows = remap[s]
        wv = rows // WIN
        cnt_all[c] = np.bincount(blk, minlength=NBLK)
        cnt1_all[c] = np.bincount(blk * NWIN + wv,
                                  minlength=NBLK * NWIN).reshape(NBLK, NWIN)
        per_core.append((s, d, blk, rows, wv))

    # L0: per-block chunk counts, common across cores
    nch0 = np.maximum(np.ceil(cnt_all.max(axis=0) / 128).astype(np.int64), 1)
    ofs0 = np.concatenate([[0], np.cumsum(nch0)])
    NCOL0 = int(ofs0[-1])
    batches0 = []
    b0 = 0
    while b0 < NBLK:
        b1 = b0 + 1
        while b1 < NBLK and b1 - b0 < 16 and ofs0[b1 + 1] - ofs0[b0] <= CMAX0:
            b1 += 1
        batches0.append((b0, b1 - b0, int(ofs0[b0]), int(ofs0[b1] - ofs0[b0])))
        b0 = b1
    assert all(bc[3] <= CMAX0 for bc in batches0)

    # L1: per-(block, window) chunk counts, common across cores
    nch1 = np.ceil(cnt1_all.max(axis=0) / 128).astype(np.int64)  # [NBLK, NWIN]
    for b in range(NBLK):
        if nch1[b].sum() == 0:
            nch1[b, 0] = 1
    blk_ch = nch1.sum(axis=1)
    assert blk_ch.max() <= CMAX1
    # group blocks into batches; assign columns window-major within batch
    batches1 = []
    col_bw = np.zeros((NBLK, NWIN), np.int64)
    col = 0
    b0 = 0
    while b0 < NBLK:
        b1 = b0 + 1
        tot = int(blk_ch[b0])
        while b1 < NBLK and b1 - b0 < 16 and tot + blk_ch[b1] <= CMAX1:
            tot += int(blk_ch[b1])
            b1 += 1
        c0 = col
        wruns = []
        blocks = {b: [] for b in range(b0, b1)}
        for w in range(NWIN):
            wc0 = col - c0
            wn = 0
            for b in range(b0, b1):
                nk = int(nch1[b, w])
                if nk:
                    col_bw[b, w] = col
                    blocks[b].append((col - c0, nk))
                    col += nk
                    wn += nk
            if wn:
                wruns.append((w, wc0, wn))
        batches1.append(dict(b0=b0, nb=b1 - b0, c0=c0, ncols=col - c0,
                             wruns=wruns,
                             blocks=[(b, blocks[b]) for b in range(b0, b1)]))
        b0 = b1
    NCOL1 = col
    assert all(bt["ncols"] <= CMAX1 for bt in batches1)

    cfg = CFG(NMAX, GMAX, nch0, ofs0, batches0, NCOL1, batches1, NWIN)

    xf = np.asarray(x, np.float32)
    x_bf = xf.astype(BFNP)
    admef = np.asarray(adme, np.float32)

    seeds = []
    for c in range(C):
        s, d, blk, rows, wv = per_core[c]
        # ---- L0 layout (block-major chunks) ----
        cnt = cnt_all[c]
        off = np.concatenate([[0], np.cumsum(cnt)])
        w_in = np.arange(len(d)) - off[blk]
        k = w_in // 128
        p = w_in % 128
        col0 = ofs0[blk] + k
        rel0 = np.full((128, NCOL0), -1.0, np.float32)
        rel0[p, col0] = (d - blk * W).astype(np.float32)
        msgs0 = np.zeros((128, NCOL0, IN_DIM), BFNP)
        msgs0[p, col0] = x_bf[s]
        msgs0 = np.ascontiguousarray(msgs0.reshape(128, NCOL0 * IN_DIM))

        # ---- L1 layout (window-major within batch) ----
        key = blk * NWIN + wv
        o2 = np.argsort(key, kind="stable")
        d2, blk2, rows2, wv2 = d[o2], blk[o2], rows[o2], wv[o2]
        grp = key[o2]
        gcnt1 = np.bincount(grp, minlength=NBLK * NWIN)
        goff1 = np.concatenate([[0], np.cumsum(gcnt1)])
        j1 = np.arange(len(d2)) - goff1[grp]
        k1 = j1 // 128
        p1 = j1 % 128
        col1 = col_bw[blk2, wv2] + k1
        rel1 = np.full((128, NCOL1), -1.0, np.float32)
        rel1[p1, col1] = (d2 - blk2 * W).astype(np.float32)
        idx1w = np.zeros((128, 8 * NCOL1), np.int16)
        idx1w[p1 % 16, 8 * col1 + p1 // 16] = (rows2 - wv2 * WIN).astype(np.int16)
        # replicate the 16-partition idx block across all 8 Q7-core stripes
        idx1w = np.ascontiguousarray(np.tile(idx1w[:16, :], (8, 1)))

        indeg = np.bincount(d, minlength=NMAX).astype(np.float32)
        inv_rep = np.ascontiguousarray(
            np.tile((1.0 / np.maximum(indeg, 1.0))[None, :], (HID, 1))).astype(BFNP)

        nv = int(ncnt[c])
        xT = np.zeros((IN_DIM, NMAX), BFNP)
        xT[:, :nv] = xf[nstart[c]:nstart[c + 1]].T.astype(BFNP)

        npad = NMAX - nv
        npad_rep = np.full((HID, 1), float(npad), np.float32)

        PC = cfg.PC
        pidx = np.full((128, PC), NMAX, np.int32)   # graph start row (NMAX -> zero tail)
        pmask = np.zeros((128, PC * 128), BFNP)     # valid slot mask per graph
        lgsz = gsz[cuts_g[c]:cuts_g[c + 1]]
        lgst = (goff[cuts_g[c]:cuts_g[c + 1]] - nstart[c]).astype(np.int64)
        for j in range(int(gcnt[c])):
            ch, pp = divmod(j, 128)
            pidx[pp, ch] = int(lgst[j])
            pmask[pp, ch * 128: ch * 128 + int(lgsz[j])] = 1.0
        inv_n = np.zeros((PC * 128, 1), np.float32)
        inv_n[:gcnt[c], 0] = 1.0 / np.maximum(lgsz, 1)

        adme_pad = np.zeros((GMAX, ADME), np.float32)
        adme_pad[:gcnt[c]] = admef[cuts_g[c]:cuts_g[c + 1]]

        seeds.append(dict(
            msgs0=msgs0, rel0=rel0.astype(BFNP), rel1=rel1.astype(BFNP),
            idx1w=idx1w,
            inv_rep=inv_rep, xT=xT, npad_rep=npad_rep,
            pidx=pidx, pmask=pmask, inv_n=inv_n, adme_pad=adme_pad,
        ))

    meta = dict(cuts_g=cuts_g, gcnt=gcnt)
    return cfg, seeds, meta


def _consts():
    cf = np.zeros((128, 256), np.float32)
    cf[:, 0:128] = np.eye(128, dtype=np.float32)
    cb = np.zeros((128, 512), np.float32)
    cb[:, CB_IOTA:CB_IOTA + 64] = np.arange(W, dtype=np.float32)[None, :]
    cb[:, CB_IDENT:CB_IDENT + 128] = np.eye(128, dtype=np.float32)
    return dict(constf_init=cf, constb_init=cb.astype(BFNP))


def _weights(ws):
    out = {}
    wb = np.zeros((128, 512), np.float32)
    wb[:IN_DIM, CB_WB:CB_WB + HID] = ws["wl0"].T
    wb[:IN_DIM, CB_WB + HID:CB_WB + 2 * HID] = ws["wr0"].T
    wb[:HID, CB_WB + 2 * HID:CB_WB + 3 * HID] = ws["wl1"].T
    wb[:HID, CB_WB + 3 * HID:CB_WB + 4 * HID] = ws["wr1"].T
    wb[:2 * HID + ADME, CB_HW1:CB_HW1 + 64] = ws["hw1"].T
    wb[:64, CB_HW2:CB_HW2 + 32] = ws["hw2"].T
    wb[:32, CB_HW3:CB_HW3 + 1] = ws["hw3"].T
    out["wtail"] = wb.astype(BFNP)  # merged into constb on device

    bn = np.zeros((128, 32), np.float32)
    for i, n in enumerate(("bc0", "g0", "be0", "bc1", "g1", "be1")):
        bn[:HID, i] = np.asarray(ws[n], np.float32)
    for i, (n, dd) in enumerate((("hb1", 64), ("hg1", 64), ("hbe1", 64),
                                 ("hb2", 32), ("hb3", 1))):
        bn[:dd, 16 + i] = np.asarray(ws[n], np.float32)
    out["bntail"] = bn
    return out


NQ = 4  # SWDGE queues; gathers round-robin across them


def build(cfg):
    nc = bacc.Bacc("TRN2", target_bir_lowering=False, debug=False, num_devices=C,
                   num_swdge_queues=NQ)
    NMAX, GMAX = cfg.NMAX, cfg.GMAX
    NCOL0, NCOL1 = cfg.NCOL0, cfg.NCOL1
    NBLK, PC, NG, NWIN = cfg.NBLK, cfg.PC, cfg.NG, cfg.NWIN
    nch0, ofs0, batches0 = cfg.nch0, cfg.ofs0, cfg.batches0
    batches1 = cfg.batches1
    FIN = 2 * HID + ADME
    RG = [list(range(C))]
    NROWS = C * NMAX

    P = nc.declare_dram_parameter
    msgs0 = P("msgs0", [128, NCOL0 * IN_DIM], BF16, isOutput=False)
    rel0 = P("rel0", [128, NCOL0], BF16, isOutput=False)
    rel1 = P("rel1", [128, NCOL1], BF16, isOutput=False)
    idx1w = P("idx1w", [128, 8 * NCOL1], I16, isOutput=False)
    inv_rep = P("inv_rep", [HID, NMAX], BF16, isOutput=False)
    xT = P("xT", [IN_DIM, NMAX], BF16, isOutput=False)
    npad_rep = P("npad_rep", [HID, 1], F32, isOutput=False)
    pidx = P("pidx", [128, PC], I32, isOutput=False)
    pmask = P("pmask", [128, PC * 128], BF16, isOutput=False)
    inv_n = P("inv_n", [PC * 128, 1], F32, isOutput=False)
    adme_pad = P("adme_pad", [GMAX, ADME], F32, isOutput=False)
    constf_init = P("constf_init", [128, 256], F32, isOutput=False)
    constb_init = P("constb_init", [128, 512], BF16, isOutput=False)
    wtail = P("wtail", [128, 512], BF16, isOutput=False)
    bntail = P("bntail", [128, 32], F32, isOutput=False)
    out_ext = P("out", [1, NG], F32, isOutput=True)

    with tile.TileContext(nc) as tc:
        with (
            tc.tile_pool(name="dram", bufs=1, space="DRAM") as dpool,
            tc.tile_pool(name="sres", bufs=1) as sres,
        ):
            tab0_loc = dpool.tile([NMAX, ROWE], BF16)
            tab0_all = dpool.tile([NROWS, ROWE], BF16, addr_space="Shared")
            tab1_loc = dpool.tile([NMAX + 128, HID], BF16)
            stat_in0 = dpool.tile([HID, 2], F32)
            stat_out0 = dpool.tile([HID, 2], F32, addr_space="Shared")
            stat_in1 = dpool.tile([HID, 2], F32)
            stat_out1 = dpool.tile([HID, 2], F32, addr_space="Shared")
            pool_in = dpool.tile([GMAX, FIN], F32)
            pool_all = dpool.tile([C * GMAX, FIN], F32, addr_space="Shared")

            nc.gpsimd.load_library(LIB_MLP)
            constf = sres.tile([128, 256], F32)
            constb = sres.tile([128, 512], BF16)
            acc = sres.tile([HID, 512], F32)  # sumacc 0:NBLK, sqacc 256:256+NBLK
            nc.sync.dma_start(out=constf[:, :], in_=constf_init[:, :])
            nc.sync.dma_start(out=constf[:, CF_BN:CF_BN + 6],
                              in_=bntail[:, 0:6])
            nc.sync.dma_start(out=constf[:, CF_HDB:CF_HDB + 5],
                              in_=bntail[:, 16:21])
            nc.sync.dma_start(out=constf[:HID, CF_BN + 6:CF_BN + 7],
                              in_=npad_rep[:, :])
            nc.sync.dma_start(out=constb[:, 0:CB_WB], in_=constb_init[:, 0:CB_WB])
            nc.sync.dma_start(out=constb[:, CB_WB:], in_=wtail[:, CB_WB:])

            def ident_f():
                return constf[:, CF_IDENT:CF_IDENT + 128]

            def bncol(i):
                return constf[:HID, CF_BN + i:CF_BN + i + 1]

            def hdbcol(i, d):
                return constf[:d, CF_HDB + i:CF_HDB + i + 1]

            def scr(i, d=HID):
                return constf[:d, CF_SCR + i:CF_SCR + i + 1]

            # ======== Layers ========
            with (
                tc.tile_pool(name="slay", bufs=1) as slay,
                tc.tile_pool(name="sio", bufs=2) as sio,
                tc.tile_pool(name="sio3", bufs=3) as sio3,
                tc.tile_pool(name="siom", bufs=2) as siom,
                tc.tile_pool(name="strs", bufs=1) as strs,
                tc.tile_pool(name="pmm", bufs=2, space="PSUM") as pmm,
                tc.tile_pool(name="phh", bufs=2, space="PSUM") as phh,
                tc.tile_pool(name="ptr", bufs=2, space="PSUM") as ptr,
            ):
                rel0_sb = slay.tile([128, NCOL0], BF16)
                rel1_sb = slay.tile([128, NCOL1], BF16)
                hpre = slay.tile([HID, NMAX], BF16)
                hpost = slay.tile([HID, NMAX], BF16)
                stage = slay.tile([128, (NMAX // 128) * ROWE], BF16)
                sqtrash = strs.tile([HID, W], F32)
                nc.sync.dma_start(out=rel0_sb[:], in_=rel0[:])
                nc.sync.dma_start(out=rel1_sb[:], in_=rel1[:])
                nc.vector.memset(stage[:], 0.0)

                def block_tail(layer, b, chain, msgs_sb, oh, invb, xTb, b_base):
                    """Aggregate chunks of block b and run the SAGE linears."""
                    Fdim = IN_DIM if layer == 0 else HID
                    estep = IN_DIM if layer == 0 else ROWE
                    wloff = CB_WB if layer == 0 else CB_WB + 2 * HID
                    bi = b - b_base
                    pa = pmm.tile([Fdim, W], F32, tag="agg")
                    nk = len(chain)
                    for i, cc in enumerate(chain):
                        nc.tensor.matmul(
                            pa[:], lhsT=msgs_sb[:, cc * estep:cc * estep + Fdim],
                            rhs=oh[:, cc * W:(cc + 1) * W],
                            start=(i == 0), stop=(i == nk - 1))
                    aggT = strs.tile([Fdim, W], BF16, tag="aggT")
                    nc.vector.tensor_tensor(
                        out=aggT[:], in0=pa[:],
                        in1=invb[:Fdim, bi * W:(bi + 1) * W],
                        op=OP.mult)
                    ph2 = phh.tile([HID, W], F32, tag="hblk")
                    nc.tensor.matmul(
                        ph2[:], lhsT=constb[:Fdim, wloff:wloff + HID],
                        rhs=aggT[:], start=True, stop=False)
                    rhs2 = (xTb[:, bi * W:(bi + 1) * W]
                            if layer == 0
                            else hpost[:, b * W:(b + 1) * W])
                    nc.tensor.matmul(
                        ph2[:], lhsT=constb[:Fdim, wloff + HID:wloff + 2 * HID],
                        rhs=rhs2, start=False, stop=True)
                    nc.scalar.activation(
                        hpre[:, b * W:(b + 1) * W], ph2[:], AF.Identity,
                        accum_out=acc[:, b:b + 1])
                    nc.scalar.activation(
                        sqtrash[:], ph2[:], AF.Square,
                        accum_out=acc[:, 256 + b:257 + b])

                def build_oh(rel_sb, c0, ncols, cmax):
                    oh = sio.tile([128, max(CMAX0, CMAX1) * W], BF16, tag="oh")
                    nc.vector.tensor_tensor(
                        out=oh[:, :ncols * W].rearrange("p (c w) -> p c w", w=W),
                        in0=rel_sb[:, c0:c0 + ncols].unsqueeze(2)
                            .to_broadcast([128, ncols, W]),
                        in1=constb[:, CB_IOTA:CB_IOTA + W].unsqueeze(1)
                            .to_broadcast([128, ncols, W]),
                        op=OP.is_equal)
                    return oh

                def sage_layer0():
                    for (b0, nb, c0, ncols) in batches0:
                        msgs = sio3.tile([128, CMAX0 * IN_DIM], BF16, tag="msgs")
                        nc.sync.dma_start(
                            out=msgs[:, :ncols * IN_DIM],
                            in_=msgs0[:, c0 * IN_DIM:(c0 + ncols) * IN_DIM])
                        xTb = sio.tile([IN_DIM, cfg.MAXB * W], BF16, tag="xTb")
                        nc.sync.dma_start(out=xTb[:, :nb * W],
                                          in_=xT[:, b0 * W:(b0 + nb) * W])
                        invb = sio.tile([HID, cfg.MAXB * W], BF16, tag="invb")
                        nc.sync.dma_start(out=invb[:, :nb * W],
                                          in_=inv_rep[:, b0 * W:(b0 + nb) * W])
                        oh = build_oh(rel0_sb, c0, ncols, CMAX0)
                        for bi in range(nb):
                            b = b0 + bi
                            lc = int(ofs0[b]) - c0
                            chain = list(range(lc, lc + int(nch0[b])))
                            block_tail(0, b, chain, msgs, oh, invb, xTb, b0)

                def sage_layer1():
                    gq = 0
                    for bt in batches1:
                        b0, nb, c0, ncols = bt["b0"], bt["nb"], bt["c0"], bt["ncols"]
                        idx_sb = sio.tile([128, 8 * CMAX1], I16, tag="idx")
                        nc.sync.dma_start(out=idx_sb[:, :8 * ncols],
                                          in_=idx1w[:, 8 * c0:8 * (c0 + ncols)])
                        msgs = siom.tile([128, CMAX1 * ROWE], BF16, tag="msgs1")
                        for (w, wc0, wn) in bt["wruns"]:
                            r0 = w * WIN
                            r1 = min(r0 + WIN, NROWS)
                            # HW caps one gather at 1024 idxs (8 columns)
                            for g0 in range(0, wn, 8):
                                gn = min(8, wn - g0)
                                ca = wc0 + g0
                                nc.gpsimd.dma_gather(
                                    out_ap=msgs[:, ca * ROWE:(ca + gn) * ROWE]
                                        .rearrange("p (c e) -> p c e", e=ROWE),
                                    in_ap=tab0_all[r0:r1, :],
                                    idxs_ap=idx_sb[:, 8 * ca:8 * (ca + gn)],
                                    num_idxs=128 * gn,
                                    num_idxs_reg=128 * gn,
                                    elem_size=ROWE,
                                    queue_num=gq % NQ)
                                gq += 1
                        invb = sio.tile([HID, cfg.MAXB * W], BF16, tag="invb")
                        nc.sync.dma_start(out=invb[:, :nb * W],
                                          in_=inv_rep[:, b0 * W:(b0 + nb) * W])
                        oh = build_oh(rel1_sb, c0, ncols, CMAX1)
                        for (b, runs) in bt["blocks"]:
                            chain = [st + k for (st, nk) in runs for k in range(nk)]
                            block_tail(1, b, chain, msgs, oh, invb, None, b0)

                def bn_params(gce, bee, corr, stat_in, stat_out):
                    ssum = scr(0)
                    nc.vector.tensor_reduce(ssum, acc[:, 0:NBLK], AX.X, OP.add)
                    ssq = scr(1)
                    nc.vector.tensor_reduce(ssq, acc[:, 256:256 + NBLK], AX.X, OP.add)
                    if corr:
                        nc.vector.tensor_tensor(ssum, ssum, constf[:HID, CF_UC:CF_UC + 1],
                                                OP.subtract)
                        nc.vector.tensor_tensor(ssq, ssq, constf[:HID, CF_UC + 1:CF_UC + 2],
                                                OP.subtract)
                    st = strs.tile([HID, 2], F32, tag="stio")
                    nc.vector.tensor_copy(st[:, 0:1], ssum)
                    nc.vector.tensor_copy(st[:, 1:2], ssq)
                    nc.sync.dma_start(out=stat_in[:], in_=st[:])
                    nc.gpsimd.collective_compute(
                        "AllReduce", OP.add, replica_groups=RG,
                        ins=[stat_in.opt()], outs=[stat_out.opt()])
                    st2 = strs.tile([HID, 2], F32, tag="stio2")
                    nc.sync.dma_start(out=st2[:], in_=stat_out[:])
                    mean = scr(2)
                    nc.vector.tensor_scalar(mean, st2[:, 0:1], 1.0 / N_NODES, None, OP.mult)
                    ex2 = scr(3)
                    nc.vector.tensor_scalar(ex2, st2[:, 1:2], 1.0 / N_NODES, None, OP.mult)
                    msq = scr(4)
                    nc.scalar.activation(msq, mean, AF.Square)
                    var = scr(5)
                    nc.vector.tensor_tensor(var, ex2, msq, OP.subtract)
                    nc.vector.tensor_scalar(var, var, EPS, None, OP.add)
                    sd = scr(6)
                    nc.scalar.activation(sd, var, AF.Sqrt)
                    rsd = scr(7)
                    nc.vector.reciprocal(rsd, sd)
                    scol = constf[:HID, CF_S:CF_S + 1]
                    tcol = constf[:HID, CF_T:CF_T + 1]
                    nc.vector.tensor_tensor(scol, gce, rsd, OP.mult)
                    tt = scr(8)
                    nc.vector.tensor_tensor(tt, mean, scol, OP.mult)
                    nc.vector.tensor_tensor(tcol, bee, tt, OP.subtract)

                def write_table(src_sb, dst_dram, rowe):
                    for i in range(NMAX // 128):
                        pt = ptr.tile([128, HID], BF16, tag="tr")
                        nc.tensor.matmul(pt[:], lhsT=src_sb[:, i * 128:(i + 1) * 128],
                                         rhs=constb[:HID, CB_IDENT:CB_IDENT + HID],
                                         is_transpose=True, start=True, stop=True)
                        nc.scalar.activation(stage[:, i * rowe:i * rowe + HID], pt[:],
                                             AF.Copy)
                    nc.sync.dma_start(
                        out=dst_dram[:].rearrange("(c p) f -> p c f", p=128)
                            [:, :NMAX // 128, :],
                        in_=stage[:, :(NMAX // 128) * rowe]
                            .rearrange("p (c f) -> p c f", f=rowe))

                # -------- Layer 0 --------
                sage_layer0()
                bn_params(bncol(1), bncol(2), False, stat_in0, stat_out0)
                nc.scalar.activation(hpost[:], hpre[:], AF.Relu,
                                     bias=constf[:HID, CF_T:CF_T + 1],
                                     scale=constf[:HID, CF_S:CF_S + 1])
                write_table(hpost, tab0_loc, ROWE)
                nc.gpsimd.collective_compute(
                    "AllGather", OP.bypass, replica_groups=RG,
                    ins=[tab0_loc.opt()], outs=[tab0_all.opt()])

                # pad-column correction for L1 stats
                vpad = scr(9)
                nc.scalar.activation(vpad, constf[:HID, CF_T:CF_T + 1], AF.Relu)
                vpad_bf = strs.tile([HID, 1], BF16, tag="vpb")
                nc.vector.tensor_copy(vpad_bf[:], vpad)
                pu = ptr.tile([HID, 1], F32, tag="pu")
                nc.tensor.matmul(pu[:], lhsT=constb[:HID, CB_WB + 3 * HID:CB_WB + 4 * HID],
                                 rhs=vpad_bf[:], start=True, stop=True)
                ucol = constf[:HID, CF_UC:CF_UC + 1]
                u2col = constf[:HID, CF_UC + 1:CF_UC + 2]
                usc = scr(10)
                nc.scalar.activation(usc, pu[:], AF.Copy)
                u2t = scr(11)
                nc.scalar.activation(u2t, pu[:], AF.Square)
                nc.vector.tensor_tensor(ucol, usc, bncol(6), OP.mult)
                nc.vector.tensor_tensor(u2col, u2t, bncol(6), OP.mult)

                # -------- Layer 1 --------
                sage_layer1()
                bn_params(bncol(4), bncol(5), True, stat_in1, stat_out1)
                nc.scalar.activation(hpost[:], hpre[:], AF.Relu,
                                     bias=constf[:HID, CF_T:CF_T + 1],
                                     scale=constf[:HID, CF_S:CF_S + 1])
                write_table(hpost, tab1_loc, HID)

            # ======== Pooling ========
            with (
                tc.tile_pool(name="spool", bufs=1) as spool,
                tc.tile_pool(name="spio", bufs=2) as spio,
            ):
                pid_sb = spool.tile([128, PC], I32)
                nc.sync.dma_start(out=pid_sb[:], in_=pidx[:])
                pmask_sb = spool.tile([128, PC * 128], BF16)
                nc.sync.dma_start(out=pmask_sb[:], in_=pmask[:])
                ztail = spool.tile([128, HID], BF16)
                nc.vector.memset(ztail[:], 0.0)
                nc.sync.dma_start(
                    out=tab1_loc[:].rearrange("(c p) f -> p c f", p=128)[:, NMAX // 128:, :],
                    in_=ztail[:].rearrange("p (c f) -> p c f", f=HID))
                invn_sb = spool.tile([128, PC], F32)
                nc.sync.dma_start(out=invn_sb[:],
                                  in_=inv_n[:].rearrange("(c p) o -> p (c o)", p=128))
                adme_sb = spool.tile([128, PC * ADME], F32)
                nc.sync.dma_start(out=adme_sb[:],
                                  in_=adme_pad[:].rearrange("(c p) f -> p c f", p=128))
                pooled = spool.tile([128, PC * FIN], F32)
                for chunk in range(PC):
                    buf = spio.tile([128, 128 * HID], BF16, tag="poolbuf")
                    nc.gpsimd.indirect_dma_start(
                        out=buf[:], out_offset=None, in_=tab1_loc[:],
                        in_offset=IndirectOffsetOnAxis(
                            ap=pid_sb[:, chunk:chunk + 1], axis=0))
                    nc.vector.tensor_tensor(
                        out=buf[:].rearrange("p (s f) -> p s f", f=HID),
                        in0=buf[:].rearrange("p (s f) -> p s f", f=HID),
                        in1=pmask_sb[:, chunk * 128:(chunk + 1) * 128].unsqueeze(2)
                            .to_broadcast([128, 128, HID]),
                        op=OP.mult)
                    half = 64 * HID
                    mx = spio.tile([128, half], BF16, tag="poolmx")
                    nc.vector.tensor_tensor(mx[:], buf[:, :half], buf[:, half:], OP.max)
                    nc.vector.tensor_tensor(buf[:, :half], buf[:, :half], buf[:, half:],
                                            OP.add)
                    sz = 32 * HID
                    while sz >= HID:
                        nc.vector.tensor_tensor(mx[:, :sz], mx[:, :sz], mx[:, sz:2 * sz],
                                                OP.max)
                        nc.vector.tensor_tensor(buf[:, :sz], buf[:, :sz], buf[:, sz:2 * sz],
                                                OP.add)
                        sz //= 2
                    off = chunk * FIN
                    nc.vector.tensor_scalar(pooled[:, off:off + HID], buf[:, :HID],
                                            invn_sb[:, chunk:chunk + 1], None, OP.mult)
                    nc.vector.tensor_copy(pooled[:, off + HID:off + 2 * HID], mx[:, :HID])
                    nc.vector.tensor_copy(pooled[:, off + 2 * HID:off + FIN],
                                          adme_sb[:, chunk * ADME:(chunk + 1) * ADME])
                nc.sync.dma_start(
                    out=pool_in[:].rearrange("(c p) f -> p c f", p=128),
                    in_=pooled[:].rearrange("p (c f) -> p c f", f=FIN))
                nc.gpsimd.collective_compute(
                    "AllGather", OP.bypass, replica_groups=RG,
                    ins=[pool_in.opt()], outs=[pool_all.opt()])

            # ======== Head (replicated) ========
            with (
                tc.tile_pool(name="shd", bufs=1) as shd,
                tc.tile_pool(name="shio", bufs=2) as shio,
                tc.tile_pool(name="phd", bufs=2, space="PSUM") as phd,
            ):
                pooledT = shd.tile([FIN, NG], BF16)
                for i in range(NG // 128):
                    pch = shio.tile([128, FIN], F32, tag="pch")
                    nc.sync.dma_start(out=pch[:], in_=pool_all[i * 128:(i + 1) * 128, :])
                    pt2 = phd.tile([FIN, 128], F32, tag="trh")
                    nc.tensor.matmul(pt2[:], lhsT=pch[:], rhs=ident_f(),
                                     is_transpose=True, start=True, stop=True)
                    nc.scalar.activation(pooledT[:, i * 128:(i + 1) * 128], pt2[:],
                                         AF.Copy)
                z1 = shd.tile([64, NG], F32)
                z1acc = shd.tile([64, 16], F32)
                sqz = shd.tile([64, 512], F32)
                for i in range(NG // 512):
                    pz = phd.tile([64, 512], F32, tag="z1")
                    nc.tensor.matmul(pz[:], lhsT=constb[:FIN, CB_HW1:CB_HW1 + 64],
                                     rhs=pooledT[:, i * 512:(i + 1) * 512],
                                     start=True, stop=True)
                    nc.scalar.activation(z1[:, i * 512:(i + 1) * 512], pz[:], AF.Identity,
                                         accum_out=z1acc[:, i:i + 1])
                    nc.scalar.activation(sqz[:], pz[:], AF.Square,
                                         accum_out=z1acc[:, 8 + i:9 + i])
                zsum = scr(12, 64)
                nc.vector.tensor_reduce(zsum, z1acc[:, 0:NG // 512], AX.X, OP.add)
                zsq = scr(13, 64)
                nc.vector.tensor_reduce(zsq, z1acc[:, 8:8 + NG // 512], AX.X, OP.add)
                zmean = scr(14, 64)
                nc.vector.tensor_scalar(zmean, zsum, 1.0 / N_GRAPHS, None, OP.mult)
                zex2 = scr(15, 64)
                nc.vector.tensor_scalar(zex2, zsq, 1.0 / N_GRAPHS, None, OP.mult)
                zmsq = scr(16, 64)
                nc.scalar.activation(zmsq, zmean, AF.Square)
                zvar = scr(17, 64)
                nc.vector.tensor_tensor(zvar, zex2, zmsq, OP.subtract)
                nc.vector.tensor_scalar(zvar, zvar, EPS, None, OP.add)
                zsd = scr(18, 64)
                nc.scalar.activation(zsd, zvar, AF.Sqrt)
                zrsd = scr(19, 64)
                nc.vector.reciprocal(zrsd, zsd)
                zs = scr(20, 64)
                nc.vector.tensor_tensor(zs, hdbcol(1, 64), zrsd, OP.mult)
                zt = scr(21, 64)
                nc.vector.tensor_tensor(zt, zmean, zs, OP.mult)
                nc.vector.tensor_tensor(zt, hdbcol(2, 64), zt, OP.subtract)
                z1b = shd.tile([64, NG], BF16)
                nc.scalar.activation(z1b[:], z1[:], AF.Relu, bias=zt, scale=zs)
                z2b = shd.tile([32, NG], BF16)
                for i in range(NG // 512):
                    pz2 = phd.tile([32, 512], F32, tag="z2")
                    nc.tensor.matmul(pz2[:], lhsT=constb[:64, CB_HW2:CB_HW2 + 32],
                                     rhs=z1b[:, i * 512:(i + 1) * 512],
                                     start=True, stop=True)
                    nc.scalar.activation(z2b[:, i * 512:(i + 1) * 512], pz2[:], AF.Relu,
                                         bias=hdbcol(3, 32))
                for i in range(NG // 512):
                    pz3 = phd.tile([1, 512], F32, tag="z3")
                    nc.tensor.matmul(pz3[:], lhsT=constb[:32, CB_HW3:CB_HW3 + 1],
                                     rhs=z2b[:, i * 512:(i + 1) * 512],
                                     start=True, stop=True)
                    zch = shio.tile([1, 512], F32, tag="zch")
                    nc.vector.tensor_scalar(zch[:], pz3[:], hdbcol(4, 1), None, OP.add)
                    nc.sync.dma_start(out=out_ext[:, i * 512:(i + 1) * 512], in_=zch[:])

    nc.compile()
    return nc


def kernel(**inputs):
    x = np.asarray(inputs["x"])
    edge_index = np.asarray(inputs["edge_index"])
    batch = np.asarray(inputs["batch"])
    adme = np.asarray(inputs["adme"])
    cfg, seeds, meta = _prep(x, edge_index, batch, adme)
    consts = _consts()
    ws = _weights(inputs)
    nc = build(cfg)
    in_maps = []
    for c in range(C):
        m = dict(seeds[c])
        m.update(consts)
        m.update(ws)
        in_maps.append(m)
    res = run_bass_kernel_spmd(nc, in_maps, core_ids=list(range(C)))
    global LAST_RESULTS
    LAST_RESULTS = res
    z = res.results[0]["out"][0]
    out = np.empty(N_GRAPHS, np.float32)
    cuts_g, gcnt = meta["cuts_g"], meta["gcnt"]
    for c in range(C):
        out[cuts_g[c]:cuts_g[c + 1]] = z[c * cfg.GMAX: c * cfg.GMAX + gcnt[c]]
    return out
